# revision 40
# baseline (speedup 1.0000x reference)
"""Trainium2 Bass kernel for nn_BRC_17179869451 (BRC-style RNN).

  xz/xr/xh = x @ {kz,kr,kh}   (three [B*T,D]x[D,H] GEMMs)
  scan over T:
      r = tanh(xr_t + h*mr + br) + 1
      z = sigmoid(xz_t + h*mz + bz)
      h = z*h + (1-z)*tanh(xh_t + r*h)

Sharding (8 cores = 8 time-segments, all 64 batches per core): the BRC
forget gate makes h_t depend only weakly on the distant past, so each
core computes a 64-step time segment for all 64 batches, preceded by a
W=24-step redundant warmup from h=0.  Segment 0 zero-pads its warmup
input, which keeps h exactly 0.

Everything on-device runs fp16: fp16 GEMMs, fp16 scan ops (DVE 2x perf
mode), fp16 output staged via the xbar DMA-transpose and upcast to
fp32 on the host.  Wide [128,512] ops amortize per-instruction
overhead.  Chunks: two 4-step head chunks (short time-to-first-step),
then 8-step chunks.  GEMM epilogues (PSUM->SBUF cast+affine) mostly run
as [128,1024] hb-pairs on ACT in each step's post-sigmoid window; the
last h-gate pair runs as two singles on DVE late in the chunk so the
next chunk's first steps never wait on ACT's epilogue tail.

Per-step math (fast path mz=mr=1; hh = h+1 shifted state, hm = h):
  s = sigmoid(2*(xr-1 + hh))            r = 2s
  q = sigmoid(4*(hm*s + xh/2))          tanh(xh + r*h) = 2q-1
  z = sigmoid(xz + hm)
  hh' = 2q(1-z) + hh*z ;  ys = hm' = hh' - 1
Layout per core: state [128 x 512]: partition h_a = h mod 128, free
(hb = h div 128 [8], b [64]).  Output: per 8-step chunk the hm ring
[128, (t,j,u)] is xbar-transposed to [u, (t,j), h_a] and DMA'd to
ys[l,b,t,j,c]; host reassembles to [b, t, h].
"""

import os
import numpy as np

B, T, D, H = 64, 512, 512, 1024
NCORES = 8
ST = 8                    # time segments
SB = 1                    # batch shards
BC = B // SB              # 64 batches per core
SEG = T // ST             # 64 output steps per core
W = 24                    # warmup steps
N = SEG + W               # 88 steps computed per core
CSIZES = [8] * 11                    # per-chunk step counts
COFFS = [sum(CSIZES[:i]) for i in range(len(CSIZES))]
NCH = len(CSIZES)         # 11 chunks
OC0 = 3                   # first output chunk (step 24)
TCO = 8                   # steps per output chunk
HB = H // 128             # 8 h-blocks
P = HB * BC               # 512 = free size of scan state
KT = D // 128             # 4 k-tiles
LJ = 128 // BC            # h-blocks packed per 128-partition u-group (2)
JD = P // 128             # u-groups per step; j-dim of ys (4)

_cache = {}


def _apply_tile_drain_patch():
    """Spread end-of-kernel sem waits over single-wait sync nops: walrus
    CoreV3 codegen rejects the stock Tile exit Drain that carries one wait
    per logical proc ("Too many sync wait commands")."""
    import concourse.tile as tile_mod

    if getattr(tile_mod.TileContext, "_drain_patched", False):
        return

    def _patched(self, tick_clock, wait_clock):
        from concourse.vector_clock import ScopedClock

        vclock = tick_clock.global_clock
        pend = [(p, vclock[p]) for p in range(len(vclock)) if vclock[p] > 0]
        for proc, tick in pend:
            sub = ScopedClock()
            sub.require_at_least(None, proc, tick)
            nop_inst = self.nc.sync.nop(nofuse=True)
            wait_clock.add_sem_waits(nop_inst.ins, sub)
        self.nc.sync.drain()
        self.nc.all_engine_barrier()
        assert self.sems is not None
        popped = self.nc._tile_sem_poison_stack.pop()
        assert popped is self._sem_poison
        self.nc.clear_and_free_semaphores(list(self.sems.allocated().values()))
        self.nc.all_engine_barrier()

    tile_mod.TileContext._drain_and_barrier = _patched
    tile_mod.TileContext._drain_patched = True


def _legalize_sync_waits(nc, max_waits: int = 1):
    """walrus codegen here rejects instructions with >1 sem wait ("Too many
    sync wait commands"); hoist extra waits onto same-engine NoOps."""
    import concourse.mybir as mybir

    n = 0
    for f in nc.m.functions:
        for bb in f.blocks:
            out = []
            for ins in bb.instructions:
                si = ins.sync_info
                if si is not None and si.on_wait and len(si.on_wait) > max_waits:
                    waits = list(si.on_wait)
                    for w in waits[:-max_waits]:
                        n += 1
                        nop = mybir.InstNoOp(
                            name=f"waitnop_{n}", engine=ins.engine)
                        nop.sync_info = mybir.SyncInfo(
                            on_wait=[w], on_update=[])
                        out.append(nop)
                    si.on_wait = waits[-max_waits:]
                out.append(ins)
            bb.instructions = out


def _build(fast: bool):
    import concourse.bass as bass
    import concourse.mybir as mybir
    from concourse.tile import TileContext

    _apply_tile_drain_patch()

    fp16 = mybir.dt.float16
    fp32 = mybir.dt.float32
    AF = mybir.ActivationFunctionType
    OP = mybir.AluOpType

    nc = bass.Bass()
    # const AP for the fast-path XR epilogue bias (br - mr = -1)
    _cb = nc.alloc_sbuf_tensor("const-f32-neg1", [128, 1], fp32)
    nc.gpsimd.memset(_cb.ap(), -1.0)
    nc.const_aps.aps[(fp32, -1.0)] = _cb.ap()
    nc.all_engine_barrier()
    xT_d = nc.dram_tensor("xT", [D, N, BC], fp16, kind="ExternalInput")
    kz_d = nc.dram_tensor("kz", [D, H], fp16, kind="ExternalInput")
    kr_d = nc.dram_tensor("kr", [D, H], fp16, kind="ExternalInput")
    kh_d = nc.dram_tensor("kh", [D, H], fp16, kind="ExternalInput")
    brv_d = nc.dram_tensor("brv", [128, HB], fp32, kind="ExternalInput")
    if not fast:
        bzv_d = nc.dram_tensor("bzv", [128, HB], fp32, kind="ExternalInput")
        mrt_d = nc.dram_tensor("mrt", [128, P], fp16, kind="ExternalInput")
        mzt_d = nc.dram_tensor("mzt", [128, P], fp16, kind="ExternalInput")
    # ys stored [l, b, t, j, c] (h = (j*LJ+l)*128+c) so the post-transpose
    # chunk DMA is perfectly linear; host reassembles to [b, t, h].
    ys_d = nc.dram_tensor("ys", [LJ, BC, SEG, JD, 128], fp16,
                          kind="ExternalOutput")

    with TileContext(nc) as tc:
        with (
            tc.tile_pool(name="const", bufs=1) as cpool,
            tc.tile_pool(name="xk", bufs=2) as xkpool,
            tc.tile_pool(name="gates", bufs=3) as gpool,
            tc.tile_pool(name="scan", bufs=3) as spool,
            tc.tile_pool(name="ring", bufs=2) as rpool,
            tc.tile_pool(name="stg", bufs=2) as stpool,
            tc.tile_pool(name="psmm", bufs=2, space="PSUM") as pspool,
            tc.tile_pool(name="psms", bufs=2, space="PSUM") as pspools,
        ):
            # ---- weight / bias tiles (DMAs emitted after chunk-0 x) ----
            w_sb = {}
            w_dma = []
            for name, wd in (("r", kr_d), ("z", kz_d), ("h", kh_d)):
                wt = cpool.tile([128, KT * H], fp16, tag=f"w{name}",
                                name=f"w{name}")
                w_dma.append((wt, wd))
                for k in range(KT):
                    w_sb[(name, k)] = wt[:, k * H:(k + 1) * H]
            brv = cpool.tile([128, HB], fp32, tag="brv", name="brv")
            if not fast:
                bzv = cpool.tile([128, HB], fp32, tag="bzv", name="bzv")
                nc.sync.dma_start(out=bzv, in_=bzv_d[:, :])
                mrt = cpool.tile([128, P], fp16, tag="mrt", name="mrt")
                nc.sync.dma_start(out=mrt, in_=mrt_d[:, :])
                mzt = cpool.tile([128, P], fp16, tag="mzt", name="mzt")
                nc.sync.dma_start(out=mzt, in_=mzt_d[:, :])

            hh0 = cpool.tile([128, P], fp16, tag="hh0", name="hh0")
            nc.vector.memset(hh0, 1.0)   # hh = h+1, h0 = 0
            hm0 = cpool.tile([128, P], fp16, tag="hm0", name="hm0")
            nc.vector.memset(hm0, 0.0)

            import bass_rust as _br

            _last = {}

            def _pin(eng, bi):
                # Pin each engine's stream to emission order; prevents
                # scheduler priority inversions (engines execute in-order).
                if eng in _last:
                    _br.add_dep_helper(bi.ins, _last[eng].ins, sync=False,
                                       reason=f"{eng} emission order")
                _last[eng] = bi
                return bi

            def vop(bi):
                return _pin("v", bi)

            def aop(bi):
                return _pin("a", bi)

            def gop(bi):
                return _pin("g", bi)

            def pe(bi):
                return _pin("pe", bi)

            # ---- GEMM pieces per chunk ----
            gates = {}   # ci -> (XR, XZ, XH) sbuf tiles [128, tc*P] fp16

            def make_pieces(ci):
                """Returns (loads, eps_act, eps_dve): closures for chunk ci's
                x load and (gate,hb) matmul+epilogue groups split by the
                engine that runs the epilogue."""
                tcc = CSIZES[ci]
                cb = tcc * BC
                co = COFFS[ci] * BC
                # pool tags need constant shapes: allocate steady-size,
                # slice for the short head chunks
                XR = gpool.tile([128, TCO * P], fp16, tag="XR",
                                name=f"XR{ci}")[:, :tcc * P]
                XZ = gpool.tile([128, TCO * P], fp16, tag="XZ",
                                name=f"XZ{ci}")[:, :tcc * P]
                XH = gpool.tile([128, TCO * P], fp16, tag="XH",
                                name=f"XH{ci}")[:, :tcc * P]
                gates[ci] = (XR, XZ, XH)
                xk = [xkpool.tile([128, TCO * BC], fp16, tag=f"xk{k}",
                                  name=f"xk{k}_{ci}")[:, :cb]
                      for k in range(KT)]

                def load(k, xk=xk):
                    # 2D view: one contiguous run per partition
                    nc.sync.dma_start(
                        out=xk[k],
                        in_=xT_d.rearrange("(k p) n b -> k p (n b)", p=128)
                        [k, :, co:co + cb])
                loads = [lambda k=k: load(k) for k in range(KT)]

                def scale_bias(g):
                    scale = 0.5 if g == "h" else 1.0
                    if g == "r":
                        bias = -1.0 if fast else brv
                    elif g == "z":
                        bias = 0.0 if fast else bzv
                    else:
                        bias = 0.0
                    return scale, bias

                def mmquad(g, hb, dest, on_dve=False, tcc=tcc, cb=cb, xk=xk,
                           ci=ci):
                    # four hb-quarters into one 4-bank psum tile, ONE wide
                    # epilogue (fast path: scale/bias identical per quarter)
                    ps = pspool.tile([128, 4 * TCO * BC], fp32, tag="mm",
                                     name=f"mm{ci}_{g}{hb}")
                    for qr in range(4):
                        for k in range(KT):
                            pe(nc.tensor.matmul(
                                out=ps[:, qr * cb:qr * cb + cb],
                                lhsT=w_sb[(g, k)][:, (hb + qr) * 128:
                                                  (hb + qr + 1) * 128],
                                rhs=xk[k],
                                start=(k == 0), stop=(k == KT - 1)))
                    dst = dest.rearrange(
                        "p (t hb b) -> p hb t b", t=tcc, hb=HB)[:, hb:hb + 4]
                    ps4 = ps[:, :4 * cb].rearrange(
                        "p (i t b) -> p i t b", i=4, t=tcc)
                    scale, bias = scale_bias(g)
                    if on_dve:   # chunk-0 priming: keep ACT free for s_0
                        sc2 = bias if isinstance(bias, float) else 0.0
                        vop(nc.vector.tensor_scalar(
                            out=dst, in0=ps4, scalar1=scale, scalar2=sc2,
                            op0=OP.mult, op1=OP.add))
                    else:
                        aop(nc.scalar.activation(
                            out=dst, in_=ps4, func=AF.Identity,
                            bias=bias, scale=scale))

                def mmsingle(g, hb, dest, on_dve, tcc=tcc, cb=cb, xk=xk,
                             ci=ci):
                    ps = pspools.tile([128, TCO * BC], fp32, tag="mms",
                                      name=f"mms{ci}_{g}{hb}")
                    for k in range(KT):
                        pe(nc.tensor.matmul(
                            out=ps[:, :cb],
                            lhsT=w_sb[(g, k)][:, hb * 128:(hb + 1) * 128],
                            rhs=xk[k],
                            start=(k == 0), stop=(k == KT - 1)))
                    dst = dest.rearrange(
                        "p (t hb b) -> p t hb b", t=tcc, hb=HB)[:, :, hb, :]
                    ps3 = ps[:, :cb].rearrange("p (t b) -> p t b", t=tcc)
                    scale, bias = scale_bias(g)
                    if g == "r" and not fast:
                        bias = brv[:, hb:hb + 1]
                    elif g == "z" and not fast:
                        bias = bzv[:, hb:hb + 1]
                    if on_dve:
                        sc2 = bias if isinstance(bias, float) else 0.0
                        vop(nc.vector.tensor_scalar(
                            out=dst, in0=ps3, scalar1=scale, scalar2=sc2,
                            op0=OP.mult, op1=OP.add))
                    else:
                        aop(nc.scalar.activation(
                            out=dst, in_=ps3, func=AF.Identity,
                            bias=bias, scale=scale))

                eps_act = []
                eps_dve = []
                if tcc < TCO or not fast:
                    # head / general chunks: all singles; h-gate on DVE in
                    # the fast path to keep ACT's queue short at startup
                    for g, dest in (("r", XR), ("z", XZ), ("h", XH)):
                        for hb in range(HB):
                            dve = fast and g == "h"
                            (eps_dve if dve else eps_act).append(
                                lambda g=g, hb=hb, dest=dest, dve=dve:
                                mmsingle(g, hb, dest, dve))
                else:
                    # steady chunks: 6 quad-epilogues (ACT; chunk-0 priming
                    # overrides z/h onto DVE via on_dve)
                    for g, dest in (("r", XR), ("z", XZ), ("h", XH)):
                        for hb in range(0, HB, 4):
                            eps_act.append(
                                lambda g=g, hb=hb, dest=dest, on_dve=False:
                                mmquad(g, hb, dest, on_dve=on_dve))
                return loads, eps_act, eps_dve

            def emit_output(ci, ring, lo, hi):
                """xbar-transpose steps [lo,hi) of chunk ci's hm ring and
                DMA to ys."""
                nt = hi - lo
                stg = stpool.tile([128, nt * P], fp16, tag="stg",
                                  name=f"stg{ci}_{lo}")
                nc.sync.dma_start_transpose(
                    out=stg.rearrange("p (g m) -> p g m", m=128),
                    in_=ring[:, lo * P:hi * P].rearrange(
                        "p (g u) -> p g u", u=128))
                ot0 = COFFS[ci] - W + lo
                dst = ys_d[:, :, ot0:ot0 + nt, :, :].rearrange(
                    "l b t j c -> (l b) t j c")
                nc.sync.dma_start(
                    out=dst,
                    in_=stg.rearrange("p (t j c) -> p t j c", t=nt, j=JD))

            # ---- emit: prime chunk 0 (x first, then weights, then r-gate
            # epilogues on ACT and z/h on DVE so nothing queues ahead of the
            # first sigmoids) ----
            pend_act = []
            pend_dve = []
            loads0, eps_act0, eps_dve0 = make_pieces(0)
            for p_ in loads0:
                p_()
            for wt, wd in w_dma:
                nc.sync.dma_start(
                    out=wt.rearrange("p (k h) -> p k h", k=KT),
                    in_=wd.rearrange("(k p) h -> p k h", p=128))
            if not fast:
                nc.sync.dma_start(out=brv, in_=brv_d[:, :])
            if fast:
                NP = HB // 4          # quad-groups per gate (2)
                for p_ in eps_act0[:NP]:   # r-quads -> ACT (feed s_0)
                    p_()
                for p_ in eps_act0[NP:]:   # z/h-quads -> DVE
                    p_(on_dve=True)
            else:
                for p_ in eps_act0:
                    p_()
            for p_ in eps_dve0:
                p_()

            hh, hm = hh0, hm0

            def s_tile(tag, i):
                return spool.tile([128, P], fp16, tag=tag, name=f"{tag}_{i}")

            def gate_col(gt, t):
                return gt[:, t * P:(t + 1) * P]

            # a_0 / c_0
            XR, XZ, XH = gates[0]
            a_t = s_tile("a", 0)
            c_t = s_tile("c", 0)
            if fast:
                vop(nc.vector.tensor_tensor(a_t, gate_col(XR, 0), hh, OP.add))
                vop(nc.vector.tensor_tensor(c_t, gate_col(XZ, 0), hm, OP.add))
            else:
                t1 = s_tile("t1", 0)
                vop(nc.vector.tensor_tensor(t1, mrt, hm, OP.mult))
                vop(nc.vector.tensor_tensor(a_t, t1, gate_col(XR, 0), OP.add))
                t2 = s_tile("t2", 0)
                vop(nc.vector.tensor_tensor(t2, mzt, hm, OP.mult))
                vop(nc.vector.tensor_tensor(c_t, t2, gate_col(XZ, 0), OP.add))

            for ci in range(NCH):
                tcc = CSIZES[ci]
                XR, XZ, XH = gates[ci]
                if ci + 1 < NCH:
                    loads, eps_a, eps_d = make_pieces(ci + 1)
                    for p_ in loads:
                        p_()
                    pend_act.extend(eps_a)
                    pend_dve.extend(eps_d)
                nsteps_left = tcc
                ring = (rpool.tile([128, TCO * P], fp16, tag="ring",
                                   name=f"ring{ci}") if ci >= OC0 else None)
                for t in range(tcc):
                    i = COFFS[ci] + t
                    last = (i == N - 1)
                    na = -(-len(pend_act) // nsteps_left)
                    nd = (-(-len(pend_dve) // nsteps_left)
                          if len(pend_dve) >= nsteps_left else 0)
                    nsteps_left -= 1
                    # chain front: s, sh, e3, q  (a_t from previous tail)
                    s_ = s_tile("s", i)
                    aop(nc.scalar.activation(s_, a_t, AF.Sigmoid, scale=2.0))
                    sh = s_tile("sh", i)
                    vop(nc.vector.tensor_tensor(sh, hm, s_, OP.mult))
                    e3 = s_tile("e3", i)
                    vop(nc.vector.tensor_tensor(e3, sh, gate_col(XH, t),
                                                OP.add))
                    z_ = s_tile("z", i)
                    aop(nc.scalar.activation(z_, c_t, AF.Sigmoid))
                    q_ = s_tile("q", i)
                    aop(nc.scalar.activation(q_, e3, AF.Sigmoid, scale=4.0))
                    # epilogue groups, in ACT's post-q window
                    for _ in range(na):
                        if pend_act:
                            pend_act.pop(0)()
                    for _ in range(nd):
                        if pend_dve:
                            pend_dve.pop(0)()
                    # off-chain tail
                    U2 = s_tile("U2", i)
                    vop(nc.vector.tensor_scalar(
                        out=U2, in0=z_, scalar1=-2.0, scalar2=2.0,
                        op0=OP.mult, op1=OP.add))
                    hz1 = s_tile("hz1", i)
                    vop(nc.vector.tensor_tensor(hz1, hh, z_, OP.mult))
                    v_ = s_tile("vv", i)
                    vop(nc.vector.tensor_tensor(v_, q_, U2, OP.mult))
                    hh_n = s_tile("hh", i)
                    vop(nc.vector.tensor_tensor(hh_n, v_, hz1, OP.add))
                    # chain-critical a' right after hh'
                    if not last and fast:
                        a_t = s_tile("a", i + 1)
                        XRn = gates[ci + 1][0] if t == tcc - 1 else XR
                        vop(nc.vector.tensor_tensor(
                            a_t, hh_n, gate_col(XRn, (t + 1) % tcc
                                                if t == tcc - 1 else t + 1),
                            OP.add))
                    hm_n = (ring[:, t * P:(t + 1) * P] if ring is not None
                            else s_tile("hm", i))
                    vop(nc.vector.tensor_scalar(
                        out=hm_n, in0=hh_n, scalar1=-1.0, scalar2=None,
                        op0=OP.add))
                    if not last:
                        c_t = s_tile("c", i + 1)
                        tn = 0 if t == tcc - 1 else t + 1
                        if fast:
                            XZn = gates[ci + 1][1] if t == tcc - 1 else XZ
                            vop(nc.vector.tensor_tensor(
                                c_t, gate_col(XZn, tn), hm_n, OP.add))
                        else:
                            a_t = s_tile("a", i + 1)
                            XRn = gates[ci + 1][0] if t == tcc - 1 else XR
                            XZn = gates[ci + 1][1] if t == tcc - 1 else XZ
                            t1 = s_tile("t1", i + 1)
                            vop(nc.vector.tensor_tensor(t1, mrt, hm_n,
                                                        OP.mult))
                            vop(nc.vector.tensor_tensor(
                                a_t, t1, gate_col(XRn, tn), OP.add))
                            t2 = s_tile("t2", i + 1)
                            gop(nc.gpsimd.tensor_tensor(t2, mzt, hm_n,
                                                        OP.mult))
                            gop(nc.gpsimd.tensor_tensor(
                                c_t, t2, gate_col(XZn, tn), OP.add))
                    hh = hh_n
                    hm = hm_n
                    # split the last chunk's output to shorten the tail
                    if (ring is not None and ci == NCH - 1
                            and t == tcc // 2 - 1):
                        emit_output(ci, ring, 0, tcc // 2)
                if ring is not None:
                    if ci == NCH - 1:
                        emit_output(ci, ring, tcc // 2, tcc)
                    else:
                        emit_output(ci, ring, 0, tcc)

    _legalize_sync_waits(nc)
    return nc


def _get_nc(fast: bool):
    if fast not in _cache:
        _cache[fast] = _build(fast)
    return _cache[fast]


LAST_RESULT = None


def kernel(**inputs):
    global LAST_RESULT
    from concourse.bass_utils import run_bass_kernel_spmd

    x = np.asarray(inputs["x"], dtype=np.float32)
    kz = np.asarray(inputs["kz"], dtype=np.float32)
    kr = np.asarray(inputs["kr"], dtype=np.float32)
    kh = np.asarray(inputs["kh"], dtype=np.float32)
    mz = np.asarray(inputs["mz"], dtype=np.float32)
    mr = np.asarray(inputs["mr"], dtype=np.float32)
    br = np.asarray(inputs["br"], dtype=np.float32)
    bz = np.asarray(inputs["bz"], dtype=np.float32)
    assert x.shape == (B, T, D) and kz.shape == (D, H)

    fast = bool(np.all(mz == 1.0) and np.all(mr == 1.0)
                and np.all(br == 0.0) and np.all(bz == 0.0)
                and not int(os.environ.get("FORCE_SLOW", "0")))
    nc = _get_nc(fast)

    def pvec(v):  # [H] -> [128, HB] with [h_a, h_b]
        return np.ascontiguousarray(v.reshape(HB, 128).T)

    def ptile(v):  # [H] -> [128, (hb, b)] fp16, replicated over b
        t = v.reshape(HB, 128).T
        return np.ascontiguousarray(
            np.repeat(t[:, :, None], BC, axis=2).reshape(128, P)
        ).astype(np.float16)

    base = {
        "kz": np.ascontiguousarray(kz).astype(np.float16),
        "kr": np.ascontiguousarray(kr).astype(np.float16),
        "kh": np.ascontiguousarray(kh).astype(np.float16),
        "brv": pvec((br - mr) if fast else br).astype(np.float32),
    }
    if not fast:
        base["bzv"] = pvec(bz).astype(np.float32)
        base["mrt"] = ptile(mr)
        base["mzt"] = ptile(mz)

    x16 = x.astype(np.float16)
    in_maps = []
    for i in range(NCORES):
        i_t, i_b = i // SB, i % SB
        t0 = i_t * SEG
        bs = slice(i_b * BC, (i_b + 1) * BC)
        xc = np.zeros((BC, N, D), np.float16)
        src = x16[bs, max(0, t0 - W):t0 + SEG]
        xc[:, N - src.shape[1]:, :] = src
        xTc = np.ascontiguousarray(xc.transpose(2, 1, 0))
        in_maps.append(dict(base, xT=xTc))

    trace = bool(int(os.environ.get("KERNEL_TRACE", "0")))
    res = run_bass_kernel_spmd(nc, in_maps, list(range(NCORES)), trace=trace)
    LAST_RESULT = res
    ys = np.empty((B, T, H), np.float32)
    for i in range(NCORES):
        i_t, i_b = i // SB, i % SB
        yc = res.results[i]["ys"].astype(np.float32)  # [l, b, t, j, c]
        ys[i_b * BC:(i_b + 1) * BC, i_t * SEG:(i_t + 1) * SEG, :] = (
            yc.transpose(1, 2, 3, 0, 4).reshape(BC, SEG, H))
    return ys


# revision 41
# speedup vs baseline: 1.0004x; 1.0004x over previous
"""Trainium2 Bass kernel for nn_BRC_17179869451 (BRC-style RNN).

  xz/xr/xh = x @ {kz,kr,kh}   (three [B*T,D]x[D,H] GEMMs)
  scan over T:
      r = tanh(xr_t + h*mr + br) + 1
      z = sigmoid(xz_t + h*mz + bz)
      h = z*h + (1-z)*tanh(xh_t + r*h)

Sharding (8 cores = 8 time-segments, all 64 batches per core): the BRC
forget gate makes h_t depend only weakly on the distant past, so each
core computes a 64-step time segment for all 64 batches, preceded by a
W=24-step redundant warmup from h=0.  Segment 0 zero-pads its warmup
input, which keeps h exactly 0.

Everything on-device runs fp16: fp16 GEMMs, fp16 scan ops (DVE 2x perf
mode), fp16 output staged via the xbar DMA-transpose and upcast to
fp32 on the host.  Wide [128,512] ops amortize per-instruction
overhead.  Chunks: two 4-step head chunks (short time-to-first-step),
then 8-step chunks.  GEMM epilogues (PSUM->SBUF cast+affine) mostly run
as [128,1024] hb-pairs on ACT in each step's post-sigmoid window; the
last h-gate pair runs as two singles on DVE late in the chunk so the
next chunk's first steps never wait on ACT's epilogue tail.

Per-step math (fast path mz=mr=1; hh = h+1 shifted state, hm = h):
  s = sigmoid(2*(xr-1 + hh))            r = 2s
  q = sigmoid(4*(hm*s + xh/2))          tanh(xh + r*h) = 2q-1
  z = sigmoid(xz + hm)
  hh' = 2q(1-z) + hh*z ;  ys = hm' = hh' - 1
Layout per core: state [128 x 512]: partition h_a = h mod 128, free
(hb = h div 128 [8], b [64]).  Output: per 8-step chunk the hm ring
[128, (t,j,u)] is xbar-transposed to [u, (t,j), h_a] and DMA'd to
ys[l,b,t,j,c]; host reassembles to [b, t, h].
"""

import os
import numpy as np

B, T, D, H = 64, 512, 512, 1024
NCORES = 8
ST = 8                    # time segments
SB = 1                    # batch shards
BC = B // SB              # 64 batches per core
SEG = T // ST             # 64 output steps per core
W = 24                    # warmup steps
N = SEG + W               # 88 steps computed per core
CSIZES = [8] * 11                    # per-chunk step counts
COFFS = [sum(CSIZES[:i]) for i in range(len(CSIZES))]
NCH = len(CSIZES)         # 11 chunks
OC0 = 3                   # first output chunk (step 24)
TCO = 8                   # steps per output chunk
HB = H // 128             # 8 h-blocks
P = HB * BC               # 512 = free size of scan state
KT = D // 128             # 4 k-tiles
LJ = 128 // BC            # h-blocks packed per 128-partition u-group (2)
JD = P // 128             # u-groups per step; j-dim of ys (4)

_cache = {}


def _apply_tile_drain_patch():
    """Spread end-of-kernel sem waits over single-wait sync nops: walrus
    CoreV3 codegen rejects the stock Tile exit Drain that carries one wait
    per logical proc ("Too many sync wait commands")."""
    import concourse.tile as tile_mod

    if getattr(tile_mod.TileContext, "_drain_patched", False):
        return

    def _patched(self, tick_clock, wait_clock):
        from concourse.vector_clock import ScopedClock

        vclock = tick_clock.global_clock
        pend = [(p, vclock[p]) for p in range(len(vclock)) if vclock[p] > 0]
        for proc, tick in pend:
            sub = ScopedClock()
            sub.require_at_least(None, proc, tick)
            nop_inst = self.nc.sync.nop(nofuse=True)
            wait_clock.add_sem_waits(nop_inst.ins, sub)
        self.nc.sync.drain()
        self.nc.all_engine_barrier()
        assert self.sems is not None
        popped = self.nc._tile_sem_poison_stack.pop()
        assert popped is self._sem_poison
        self.nc.clear_and_free_semaphores(list(self.sems.allocated().values()))
        self.nc.all_engine_barrier()

    tile_mod.TileContext._drain_and_barrier = _patched
    tile_mod.TileContext._drain_patched = True


def _legalize_sync_waits(nc, max_waits: int = 1):
    """walrus codegen here rejects instructions with >1 sem wait ("Too many
    sync wait commands"); hoist extra waits onto same-engine NoOps."""
    import concourse.mybir as mybir

    n = 0
    for f in nc.m.functions:
        for bb in f.blocks:
            out = []
            for ins in bb.instructions:
                si = ins.sync_info
                if si is not None and si.on_wait and len(si.on_wait) > max_waits:
                    waits = list(si.on_wait)
                    for w in waits[:-max_waits]:
                        n += 1
                        nop = mybir.InstNoOp(
                            name=f"waitnop_{n}", engine=ins.engine)
                        nop.sync_info = mybir.SyncInfo(
                            on_wait=[w], on_update=[])
                        out.append(nop)
                    si.on_wait = waits[-max_waits:]
                out.append(ins)
            bb.instructions = out


def _build(fast: bool):
    import concourse.bass as bass
    import concourse.mybir as mybir
    from concourse.tile import TileContext

    _apply_tile_drain_patch()

    fp16 = mybir.dt.float16
    fp32 = mybir.dt.float32
    AF = mybir.ActivationFunctionType
    OP = mybir.AluOpType

    nc = bass.Bass()
    # const AP for the fast-path XR epilogue bias (br - mr = -1)
    _cb = nc.alloc_sbuf_tensor("const-f32-neg1", [128, 1], fp32)
    nc.gpsimd.memset(_cb.ap(), -1.0)
    nc.const_aps.aps[(fp32, -1.0)] = _cb.ap()
    nc.all_engine_barrier()
    xT_d = nc.dram_tensor("xT", [D, N, BC], fp16, kind="ExternalInput")
    kz_d = nc.dram_tensor("kz", [D, H], fp16, kind="ExternalInput")
    kr_d = nc.dram_tensor("kr", [D, H], fp16, kind="ExternalInput")
    kh_d = nc.dram_tensor("kh", [D, H], fp16, kind="ExternalInput")
    brv_d = nc.dram_tensor("brv", [128, HB], fp32, kind="ExternalInput")
    if not fast:
        bzv_d = nc.dram_tensor("bzv", [128, HB], fp32, kind="ExternalInput")
        mrt_d = nc.dram_tensor("mrt", [128, P], fp16, kind="ExternalInput")
        mzt_d = nc.dram_tensor("mzt", [128, P], fp16, kind="ExternalInput")
    # ys stored [l, b, t, j, c] (h = (j*LJ+l)*128+c) so the post-transpose
    # chunk DMA is perfectly linear; host reassembles to [b, t, h].
    ys_d = nc.dram_tensor("ys", [LJ, BC, SEG, JD, 128], fp16,
                          kind="ExternalOutput")

    with TileContext(nc) as tc:
        with (
            tc.tile_pool(name="const", bufs=1) as cpool,
            tc.tile_pool(name="xk", bufs=2) as xkpool,
            tc.tile_pool(name="gates", bufs=3) as gpool,
            tc.tile_pool(name="scan", bufs=3) as spool,
            tc.tile_pool(name="ring", bufs=2) as rpool,
            tc.tile_pool(name="stg", bufs=2) as stpool,
            tc.tile_pool(name="psmm", bufs=3, space="PSUM") as pspool,
            tc.tile_pool(name="psms", bufs=2, space="PSUM") as pspools,
        ):
            # ---- weight / bias tiles (DMAs emitted after chunk-0 x) ----
            w_sb = {}
            w_dma = []
            for name, wd in (("r", kr_d), ("h", kh_d), ("z", kz_d)):
                wt = cpool.tile([128, KT * H], fp16, tag=f"w{name}",
                                name=f"w{name}")
                w_dma.append((wt, wd))
                for k in range(KT):
                    w_sb[(name, k)] = wt[:, k * H:(k + 1) * H]
            brv = cpool.tile([128, HB], fp32, tag="brv", name="brv")
            if not fast:
                bzv = cpool.tile([128, HB], fp32, tag="bzv", name="bzv")
                nc.sync.dma_start(out=bzv, in_=bzv_d[:, :])
                mrt = cpool.tile([128, P], fp16, tag="mrt", name="mrt")
                nc.sync.dma_start(out=mrt, in_=mrt_d[:, :])
                mzt = cpool.tile([128, P], fp16, tag="mzt", name="mzt")
                nc.sync.dma_start(out=mzt, in_=mzt_d[:, :])

            hh0 = cpool.tile([128, P], fp16, tag="hh0", name="hh0")
            nc.vector.memset(hh0, 1.0)   # hh = h+1, h0 = 0
            hm0 = cpool.tile([128, P], fp16, tag="hm0", name="hm0")
            nc.vector.memset(hm0, 0.0)

            import bass_rust as _br

            _last = {}

            def _pin(eng, bi):
                # Pin each engine's stream to emission order; prevents
                # scheduler priority inversions (engines execute in-order).
                if eng in _last:
                    _br.add_dep_helper(bi.ins, _last[eng].ins, sync=False,
                                       reason=f"{eng} emission order")
                _last[eng] = bi
                return bi

            def vop(bi):
                return _pin("v", bi)

            def aop(bi):
                return _pin("a", bi)

            def gop(bi):
                return _pin("g", bi)

            def pe(bi):
                return _pin("pe", bi)

            # ---- GEMM pieces per chunk ----
            gates = {}   # ci -> (XR, XZ, XH) sbuf tiles [128, tc*P] fp16

            def make_pieces(ci):
                """Returns (loads, eps_act, eps_dve): closures for chunk ci's
                x load and (gate,hb) matmul+epilogue groups split by the
                engine that runs the epilogue."""
                tcc = CSIZES[ci]
                cb = tcc * BC
                co = COFFS[ci] * BC
                # pool tags need constant shapes: allocate steady-size,
                # slice for the short head chunks
                XR = gpool.tile([128, TCO * P], fp16, tag="XR",
                                name=f"XR{ci}")[:, :tcc * P]
                XZ = gpool.tile([128, TCO * P], fp16, tag="XZ",
                                name=f"XZ{ci}")[:, :tcc * P]
                XH = gpool.tile([128, TCO * P], fp16, tag="XH",
                                name=f"XH{ci}")[:, :tcc * P]
                gates[ci] = (XR, XZ, XH)
                xk = [xkpool.tile([128, TCO * BC], fp16, tag=f"xk{k}",
                                  name=f"xk{k}_{ci}")[:, :cb]
                      for k in range(KT)]

                def load(k, xk=xk):
                    # 2D view: one contiguous run per partition
                    nc.sync.dma_start(
                        out=xk[k],
                        in_=xT_d.rearrange("(k p) n b -> k p (n b)", p=128)
                        [k, :, co:co + cb])
                loads = [lambda k=k: load(k) for k in range(KT)]

                def scale_bias(g):
                    scale = 0.5 if g == "h" else 1.0
                    if g == "r":
                        bias = -1.0 if fast else brv
                    elif g == "z":
                        bias = 0.0 if fast else bzv
                    else:
                        bias = 0.0
                    return scale, bias

                def mmpair(g, hb, dest, on_dve=False, tcc=tcc, cb=cb, xk=xk,
                           ci=ci):
                    # two hb-halves into one psum tile, ONE wide epilogue
                    ps = pspool.tile([128, 2 * TCO * BC], fp32, tag="mm",
                                     name=f"mm{ci}_{g}{hb}")
                    for half in range(2):
                        for k in range(KT):
                            pe(nc.tensor.matmul(
                                out=ps[:, half * cb:half * cb + cb],
                                lhsT=w_sb[(g, k)][:, (hb + half) * 128:
                                                  (hb + half + 1) * 128],
                                rhs=xk[k],
                                start=(k == 0), stop=(k == KT - 1)))
                    dst = dest.rearrange(
                        "p (t hb b) -> p hb t b", t=tcc, hb=HB)[:, hb:hb + 2]
                    ps4 = ps[:, :2 * cb].rearrange(
                        "p (i t b) -> p i t b", i=2, t=tcc)
                    scale, bias = scale_bias(g)
                    if g == "r" and not fast:
                        bias = brv[:, hb:hb + 1]  # not pair-safe in general
                    if on_dve:   # chunk-0 priming: keep ACT free for s_0
                        sc2 = bias if isinstance(bias, float) else 0.0
                        vop(nc.vector.tensor_scalar(
                            out=dst, in0=ps4, scalar1=scale, scalar2=sc2,
                            op0=OP.mult, op1=OP.add))
                    else:
                        aop(nc.scalar.activation(
                            out=dst, in_=ps4, func=AF.Identity,
                            bias=bias, scale=scale))

                def mmsingle(g, hb, dest, on_dve, tcc=tcc, cb=cb, xk=xk,
                             ci=ci):
                    ps = pspools.tile([128, TCO * BC], fp32, tag="mms",
                                      name=f"mms{ci}_{g}{hb}")
                    for k in range(KT):
                        pe(nc.tensor.matmul(
                            out=ps[:, :cb],
                            lhsT=w_sb[(g, k)][:, hb * 128:(hb + 1) * 128],
                            rhs=xk[k],
                            start=(k == 0), stop=(k == KT - 1)))
                    dst = dest.rearrange(
                        "p (t hb b) -> p t hb b", t=tcc, hb=HB)[:, :, hb, :]
                    ps3 = ps[:, :cb].rearrange("p (t b) -> p t b", t=tcc)
                    scale, bias = scale_bias(g)
                    if g == "r" and not fast:
                        bias = brv[:, hb:hb + 1]
                    elif g == "z" and not fast:
                        bias = bzv[:, hb:hb + 1]
                    if on_dve:
                        sc2 = bias if isinstance(bias, float) else 0.0
                        vop(nc.vector.tensor_scalar(
                            out=dst, in0=ps3, scalar1=scale, scalar2=sc2,
                            op0=OP.mult, op1=OP.add))
                    else:
                        aop(nc.scalar.activation(
                            out=dst, in_=ps3, func=AF.Identity,
                            bias=bias, scale=scale))

                eps_act = []
                eps_dve = []
                if tcc < TCO or not fast:
                    # head / general chunks: all singles; h-gate on DVE in
                    # the fast path to keep ACT's queue short at startup
                    for g, dest in (("r", XR), ("z", XZ), ("h", XH)):
                        for hb in range(HB):
                            dve = fast and g == "h"
                            (eps_dve if dve else eps_act).append(
                                lambda g=g, hb=hb, dest=dest, dve=dve:
                                mmsingle(g, hb, dest, dve))
                else:
                    # steady chunks: 12 pair-epilogues (ACT; chunk-0 priming
                    # overrides z/h onto DVE via on_dve)
                    for g, dest in (("r", XR), ("z", XZ), ("h", XH)):
                        for hb in range(0, HB, 2):
                            eps_act.append(
                                lambda g=g, hb=hb, dest=dest, on_dve=False:
                                mmpair(g, hb, dest, on_dve=on_dve))
                return loads, eps_act, eps_dve

            def emit_output(ci, ring, lo, hi):
                """xbar-transpose steps [lo,hi) of chunk ci's hm ring and
                DMA to ys."""
                nt = hi - lo
                stg = stpool.tile([128, nt * P], fp16, tag="stg",
                                  name=f"stg{ci}_{lo}")
                nc.sync.dma_start_transpose(
                    out=stg.rearrange("p (g m) -> p g m", m=128),
                    in_=ring[:, lo * P:hi * P].rearrange(
                        "p (g u) -> p g u", u=128))
                ot0 = COFFS[ci] - W + lo
                dst = ys_d[:, :, ot0:ot0 + nt, :, :].rearrange(
                    "l b t j c -> (l b) t j c")
                nc.sync.dma_start(
                    out=dst,
                    in_=stg.rearrange("p (t j c) -> p t j c", t=nt, j=JD))

            # ---- emit: prime chunk 0 (x first, then weights, then r-gate
            # epilogues on ACT and z/h on DVE so nothing queues ahead of the
            # first sigmoids) ----
            pend_act = []
            pend_dve = []
            loads0, eps_act0, eps_dve0 = make_pieces(0)
            for p_ in loads0:
                p_()
            for wt, wd in w_dma:
                nc.sync.dma_start(
                    out=wt.rearrange("p (k h) -> p k h", k=KT),
                    in_=wd.rearrange("(k p) h -> p k h", p=128))
            if not fast:
                nc.sync.dma_start(out=brv, in_=brv_d[:, :])
            if fast:
                NP = HB // 2          # pair-groups per gate (4)
                for p_ in eps_act0[:NP]:   # r-pairs -> ACT (feed s_0)
                    p_()
                for p_ in eps_act0[NP:]:   # z/h-pairs -> DVE
                    p_(on_dve=True)
            else:
                for p_ in eps_act0:
                    p_()
            for p_ in eps_dve0:
                p_()

            hh, hm = hh0, hm0

            def s_tile(tag, i):
                return spool.tile([128, P], fp16, tag=tag, name=f"{tag}_{i}")

            def gate_col(gt, t):
                return gt[:, t * P:(t + 1) * P]

            # a_0 / c_0
            XR, XZ, XH = gates[0]
            a_t = s_tile("a", 0)
            c_t = s_tile("c", 0)
            if fast:
                vop(nc.vector.tensor_tensor(a_t, gate_col(XR, 0), hh, OP.add))
                vop(nc.vector.tensor_tensor(c_t, gate_col(XZ, 0), hm, OP.add))
            else:
                t1 = s_tile("t1", 0)
                vop(nc.vector.tensor_tensor(t1, mrt, hm, OP.mult))
                vop(nc.vector.tensor_tensor(a_t, t1, gate_col(XR, 0), OP.add))
                t2 = s_tile("t2", 0)
                vop(nc.vector.tensor_tensor(t2, mzt, hm, OP.mult))
                vop(nc.vector.tensor_tensor(c_t, t2, gate_col(XZ, 0), OP.add))

            for ci in range(NCH):
                tcc = CSIZES[ci]
                XR, XZ, XH = gates[ci]
                if ci + 1 < NCH:
                    loads, eps_a, eps_d = make_pieces(ci + 1)
                    for p_ in loads:
                        p_()
                    pend_act.extend(eps_a)
                    pend_dve.extend(eps_d)
                nsteps_left = tcc
                ring = (rpool.tile([128, TCO * P], fp16, tag="ring",
                                   name=f"ring{ci}") if ci >= OC0 else None)
                for t in range(tcc):
                    i = COFFS[ci] + t
                    last = (i == N - 1)
                    na = -(-len(pend_act) // nsteps_left)
                    nd = (-(-len(pend_dve) // nsteps_left)
                          if len(pend_dve) >= nsteps_left else 0)
                    nsteps_left -= 1
                    # chain front: s, sh, e3, q  (a_t from previous tail)
                    s_ = s_tile("s", i)
                    aop(nc.scalar.activation(s_, a_t, AF.Sigmoid, scale=2.0))
                    sh = s_tile("sh", i)
                    vop(nc.vector.tensor_tensor(sh, hm, s_, OP.mult))
                    e3 = s_tile("e3", i)
                    vop(nc.vector.tensor_tensor(e3, sh, gate_col(XH, t),
                                                OP.add))
                    z_ = s_tile("z", i)
                    aop(nc.scalar.activation(z_, c_t, AF.Sigmoid))
                    q_ = s_tile("q", i)
                    aop(nc.scalar.activation(q_, e3, AF.Sigmoid, scale=4.0))
                    # epilogue groups, in ACT's post-q window
                    for _ in range(na):
                        if pend_act:
                            pend_act.pop(0)()
                    for _ in range(nd):
                        if pend_dve:
                            pend_dve.pop(0)()
                    # off-chain tail
                    U2 = s_tile("U2", i)
                    vop(nc.vector.tensor_scalar(
                        out=U2, in0=z_, scalar1=-2.0, scalar2=2.0,
                        op0=OP.mult, op1=OP.add))
                    hz1 = s_tile("hz1", i)
                    vop(nc.vector.tensor_tensor(hz1, hh, z_, OP.mult))
                    v_ = s_tile("vv", i)
                    vop(nc.vector.tensor_tensor(v_, q_, U2, OP.mult))
                    hh_n = s_tile("hh", i)
                    vop(nc.vector.tensor_tensor(hh_n, v_, hz1, OP.add))
                    # chain-critical a' right after hh'
                    if not last and fast:
                        a_t = s_tile("a", i + 1)
                        XRn = gates[ci + 1][0] if t == tcc - 1 else XR
                        vop(nc.vector.tensor_tensor(
                            a_t, hh_n, gate_col(XRn, (t + 1) % tcc
                                                if t == tcc - 1 else t + 1),
                            OP.add))
                    hm_n = (ring[:, t * P:(t + 1) * P] if ring is not None
                            else s_tile("hm", i))
                    vop(nc.vector.tensor_scalar(
                        out=hm_n, in0=hh_n, scalar1=-1.0, scalar2=None,
                        op0=OP.add))
                    if not last:
                        c_t = s_tile("c", i + 1)
                        tn = 0 if t == tcc - 1 else t + 1
                        if fast:
                            XZn = gates[ci + 1][1] if t == tcc - 1 else XZ
                            vop(nc.vector.tensor_tensor(
                                c_t, gate_col(XZn, tn), hm_n, OP.add))
                        else:
                            a_t = s_tile("a", i + 1)
                            XRn = gates[ci + 1][0] if t == tcc - 1 else XR
                            XZn = gates[ci + 1][1] if t == tcc - 1 else XZ
                            t1 = s_tile("t1", i + 1)
                            vop(nc.vector.tensor_tensor(t1, mrt, hm_n,
                                                        OP.mult))
                            vop(nc.vector.tensor_tensor(
                                a_t, t1, gate_col(XRn, tn), OP.add))
                            t2 = s_tile("t2", i + 1)
                            gop(nc.gpsimd.tensor_tensor(t2, mzt, hm_n,
                                                        OP.mult))
                            gop(nc.gpsimd.tensor_tensor(
                                c_t, t2, gate_col(XZn, tn), OP.add))
                    hh = hh_n
                    hm = hm_n
                    # split the last chunk's output to shorten the tail
                    if (ring is not None and ci == NCH - 1
                            and t == tcc // 2 - 1):
                        emit_output(ci, ring, 0, tcc // 2)
                if ring is not None:
                    if ci == NCH - 1:
                        emit_output(ci, ring, tcc // 2, tcc)
                    else:
                        emit_output(ci, ring, 0, tcc)

    _legalize_sync_waits(nc)
    return nc


def _get_nc(fast: bool):
    if fast not in _cache:
        _cache[fast] = _build(fast)
    return _cache[fast]


LAST_RESULT = None


def kernel(**inputs):
    global LAST_RESULT
    from concourse.bass_utils import run_bass_kernel_spmd

    x = np.asarray(inputs["x"], dtype=np.float32)
    kz = np.asarray(inputs["kz"], dtype=np.float32)
    kr = np.asarray(inputs["kr"], dtype=np.float32)
    kh = np.asarray(inputs["kh"], dtype=np.float32)
    mz = np.asarray(inputs["mz"], dtype=np.float32)
    mr = np.asarray(inputs["mr"], dtype=np.float32)
    br = np.asarray(inputs["br"], dtype=np.float32)
    bz = np.asarray(inputs["bz"], dtype=np.float32)
    assert x.shape == (B, T, D) and kz.shape == (D, H)

    fast = bool(np.all(mz == 1.0) and np.all(mr == 1.0)
                and np.all(br == 0.0) and np.all(bz == 0.0)
                and not int(os.environ.get("FORCE_SLOW", "0")))
    nc = _get_nc(fast)

    def pvec(v):  # [H] -> [128, HB] with [h_a, h_b]
        return np.ascontiguousarray(v.reshape(HB, 128).T)

    def ptile(v):  # [H] -> [128, (hb, b)] fp16, replicated over b
        t = v.reshape(HB, 128).T
        return np.ascontiguousarray(
            np.repeat(t[:, :, None], BC, axis=2).reshape(128, P)
        ).astype(np.float16)

    base = {
        "kz": np.ascontiguousarray(kz).astype(np.float16),
        "kr": np.ascontiguousarray(kr).astype(np.float16),
        "kh": np.ascontiguousarray(kh).astype(np.float16),
        "brv": pvec((br - mr) if fast else br).astype(np.float32),
    }
    if not fast:
        base["bzv"] = pvec(bz).astype(np.float32)
        base["mrt"] = ptile(mr)
        base["mzt"] = ptile(mz)

    x16 = x.astype(np.float16)
    in_maps = []
    for i in range(NCORES):
        i_t, i_b = i // SB, i % SB
        t0 = i_t * SEG
        bs = slice(i_b * BC, (i_b + 1) * BC)
        xc = np.zeros((BC, N, D), np.float16)
        src = x16[bs, max(0, t0 - W):t0 + SEG]
        xc[:, N - src.shape[1]:, :] = src
        xTc = np.ascontiguousarray(xc.transpose(2, 1, 0))
        in_maps.append(dict(base, xT=xTc))

    trace = bool(int(os.environ.get("KERNEL_TRACE", "0")))
    res = run_bass_kernel_spmd(nc, in_maps, list(range(NCORES)), trace=trace)
    LAST_RESULT = res
    ys = np.empty((B, T, H), np.float32)
    for i in range(NCORES):
        i_t, i_b = i // SB, i % SB
        yc = res.results[i]["ys"].astype(np.float32)  # [l, b, t, j, c]
        ys[i_b * BC:(i_b + 1) * BC, i_t * SEG:(i_t + 1) * SEG, :] = (
            yc.transpose(1, 2, 3, 0, 4).reshape(BC, SEG, H))
    return ys


# revision 42
# speedup vs baseline: 1.0036x; 1.0032x over previous
"""Trainium2 Bass kernel for nn_BRC_17179869451 (BRC-style RNN).

  xz/xr/xh = x @ {kz,kr,kh}   (three [B*T,D]x[D,H] GEMMs)
  scan over T:
      r = tanh(xr_t + h*mr + br) + 1
      z = sigmoid(xz_t + h*mz + bz)
      h = z*h + (1-z)*tanh(xh_t + r*h)

Sharding (8 cores = 8 time-segments, all 64 batches per core): the BRC
forget gate makes h_t depend only weakly on the distant past, so each
core computes a 64-step time segment for all 64 batches, preceded by a
W=24-step redundant warmup from h=0.  Segment 0 zero-pads its warmup
input, which keeps h exactly 0.

Everything on-device runs fp16: fp16 GEMMs, fp16 scan ops (DVE 2x perf
mode), fp16 output staged via the xbar DMA-transpose and upcast to
fp32 on the host.  Wide [128,512] ops amortize per-instruction
overhead.  Chunks: two 4-step head chunks (short time-to-first-step),
then 8-step chunks.  GEMM epilogues (PSUM->SBUF cast+affine) mostly run
as [128,1024] hb-pairs on ACT in each step's post-sigmoid window; the
last h-gate pair runs as two singles on DVE late in the chunk so the
next chunk's first steps never wait on ACT's epilogue tail.

Per-step math (fast path mz=mr=1; hh = h+1 shifted state, hm = h):
  s = sigmoid(2*(xr-1 + hh))            r = 2s
  q = sigmoid(4*(hm*s + xh/2))          tanh(xh + r*h) = 2q-1
  z = sigmoid(xz + hm)
  hh' = 2q(1-z) + hh*z ;  ys = hm' = hh' - 1
Layout per core: state [128 x 512]: partition h_a = h mod 128, free
(hb = h div 128 [8], b [64]).  Output: per 8-step chunk the hm ring
[128, (t,j,u)] is xbar-transposed to [u, (t,j), h_a] and DMA'd to
ys[l,b,t,j,c]; host reassembles to [b, t, h].
"""

import os
import numpy as np

B, T, D, H = 64, 512, 512, 1024
NCORES = 8
ST = 8                    # time segments
SB = 1                    # batch shards
BC = B // SB              # 64 batches per core
SEG = T // ST             # 64 output steps per core
W = 24                    # warmup steps
N = SEG + W               # 88 steps computed per core
CSIZES = [8] * 11                    # per-chunk step counts
COFFS = [sum(CSIZES[:i]) for i in range(len(CSIZES))]
NCH = len(CSIZES)         # 11 chunks
OC0 = 3                   # first output chunk (step 24)
TCO = 8                   # steps per output chunk
HB = H // 128             # 8 h-blocks
P = HB * BC               # 512 = free size of scan state
KT = D // 128             # 4 k-tiles
LJ = 128 // BC            # h-blocks packed per 128-partition u-group (2)
JD = P // 128             # u-groups per step; j-dim of ys (4)

_cache = {}


def _apply_tile_drain_patch():
    """Spread end-of-kernel sem waits over single-wait sync nops: walrus
    CoreV3 codegen rejects the stock Tile exit Drain that carries one wait
    per logical proc ("Too many sync wait commands")."""
    import concourse.tile as tile_mod

    if getattr(tile_mod.TileContext, "_drain_patched", False):
        return

    def _patched(self, tick_clock, wait_clock):
        from concourse.vector_clock import ScopedClock

        vclock = tick_clock.global_clock
        pend = [(p, vclock[p]) for p in range(len(vclock)) if vclock[p] > 0]
        for proc, tick in pend:
            sub = ScopedClock()
            sub.require_at_least(None, proc, tick)
            nop_inst = self.nc.sync.nop(nofuse=True)
            wait_clock.add_sem_waits(nop_inst.ins, sub)
        self.nc.sync.drain()
        self.nc.all_engine_barrier()
        assert self.sems is not None
        popped = self.nc._tile_sem_poison_stack.pop()
        assert popped is self._sem_poison
        self.nc.clear_and_free_semaphores(list(self.sems.allocated().values()))
        self.nc.all_engine_barrier()

    tile_mod.TileContext._drain_and_barrier = _patched
    tile_mod.TileContext._drain_patched = True


def _legalize_sync_waits(nc, max_waits: int = 1):
    """walrus codegen here rejects instructions with >1 sem wait ("Too many
    sync wait commands"); hoist extra waits onto same-engine NoOps."""
    import concourse.mybir as mybir

    n = 0
    for f in nc.m.functions:
        for bb in f.blocks:
            out = []
            for ins in bb.instructions:
                si = ins.sync_info
                if si is not None and si.on_wait and len(si.on_wait) > max_waits:
                    waits = list(si.on_wait)
                    for w in waits[:-max_waits]:
                        n += 1
                        nop = mybir.InstNoOp(
                            name=f"waitnop_{n}", engine=ins.engine)
                        nop.sync_info = mybir.SyncInfo(
                            on_wait=[w], on_update=[])
                        out.append(nop)
                    si.on_wait = waits[-max_waits:]
                out.append(ins)
            bb.instructions = out


def _build(fast: bool):
    import concourse.bass as bass
    import concourse.mybir as mybir
    from concourse.tile import TileContext

    _apply_tile_drain_patch()

    fp16 = mybir.dt.float16
    fp32 = mybir.dt.float32
    AF = mybir.ActivationFunctionType
    OP = mybir.AluOpType

    nc = bass.Bass()
    # const AP for the fast-path XR epilogue bias (br - mr = -1)
    _cb = nc.alloc_sbuf_tensor("const-f32-neg1", [128, 1], fp32)
    nc.gpsimd.memset(_cb.ap(), -1.0)
    nc.const_aps.aps[(fp32, -1.0)] = _cb.ap()
    nc.all_engine_barrier()
    xT_d = nc.dram_tensor("xT", [D, N, BC], fp16, kind="ExternalInput")
    kz_d = nc.dram_tensor("kz", [D, H], fp16, kind="ExternalInput")
    kr_d = nc.dram_tensor("kr", [D, H], fp16, kind="ExternalInput")
    kh_d = nc.dram_tensor("kh", [D, H], fp16, kind="ExternalInput")
    brv_d = nc.dram_tensor("brv", [128, HB], fp32, kind="ExternalInput")
    if not fast:
        bzv_d = nc.dram_tensor("bzv", [128, HB], fp32, kind="ExternalInput")
        mrt_d = nc.dram_tensor("mrt", [128, P], fp16, kind="ExternalInput")
        mzt_d = nc.dram_tensor("mzt", [128, P], fp16, kind="ExternalInput")
    # ys stored [l, b, t, j, c] (h = (j*LJ+l)*128+c) so the post-transpose
    # chunk DMA is perfectly linear; host reassembles to [b, t, h].
    ys_d = nc.dram_tensor("ys", [LJ, BC, SEG, JD, 128], fp16,
                          kind="ExternalOutput")

    with TileContext(nc) as tc:
        with (
            tc.tile_pool(name="const", bufs=1) as cpool,
            tc.tile_pool(name="xk", bufs=2) as xkpool,
            tc.tile_pool(name="gates", bufs=3) as gpool,
            tc.tile_pool(name="scan", bufs=3) as spool,
            tc.tile_pool(name="ring", bufs=2) as rpool,
            tc.tile_pool(name="stg", bufs=2) as stpool,
            tc.tile_pool(name="psmm", bufs=3, space="PSUM") as pspool,
            tc.tile_pool(name="psms", bufs=2, space="PSUM") as pspools,
        ):
            # ---- weight / bias tiles (DMAs emitted after chunk-0 x) ----
            w_sb = {}
            w_dma = []
            for name, wd in (("r", kr_d), ("h", kh_d), ("z", kz_d)):
                wt = cpool.tile([128, KT * H], fp16, tag=f"w{name}",
                                name=f"w{name}")
                w_dma.append((wt, wd))
                for k in range(KT):
                    w_sb[(name, k)] = wt[:, k * H:(k + 1) * H]
            brv = cpool.tile([128, HB], fp32, tag="brv", name="brv")
            if not fast:
                bzv = cpool.tile([128, HB], fp32, tag="bzv", name="bzv")
                nc.sync.dma_start(out=bzv, in_=bzv_d[:, :])
                mrt = cpool.tile([128, P], fp16, tag="mrt", name="mrt")
                nc.sync.dma_start(out=mrt, in_=mrt_d[:, :])
                mzt = cpool.tile([128, P], fp16, tag="mzt", name="mzt")
                nc.sync.dma_start(out=mzt, in_=mzt_d[:, :])

            hh0 = cpool.tile([128, P], fp16, tag="hh0", name="hh0")
            nc.vector.memset(hh0, 1.0)   # hh = h+1, h0 = 0
            hm0 = cpool.tile([128, P], fp16, tag="hm0", name="hm0")
            nc.vector.memset(hm0, 0.0)

            import bass_rust as _br

            _last = {}

            def _pin(eng, bi):
                # Pin each engine's stream to emission order; prevents
                # scheduler priority inversions (engines execute in-order).
                if eng in _last:
                    _br.add_dep_helper(bi.ins, _last[eng].ins, sync=False,
                                       reason=f"{eng} emission order")
                _last[eng] = bi
                return bi

            def vop(bi):
                return _pin("v", bi)

            def aop(bi):
                return _pin("a", bi)

            def gop(bi):
                return _pin("g", bi)

            def pe(bi):
                return _pin("pe", bi)

            # ---- GEMM pieces per chunk ----
            gates = {}   # ci -> (XR, XZ, XH) sbuf tiles [128, tc*P] fp16

            def make_pieces(ci):
                """Returns (loads, eps_act, eps_dve): closures for chunk ci's
                x load and (gate,hb) matmul+epilogue groups split by the
                engine that runs the epilogue."""
                tcc = CSIZES[ci]
                cb = tcc * BC
                co = COFFS[ci] * BC
                # pool tags need constant shapes: allocate steady-size,
                # slice for the short head chunks
                XR = gpool.tile([128, TCO * P], fp16, tag="XR",
                                name=f"XR{ci}")[:, :tcc * P]
                XZ = gpool.tile([128, TCO * P], fp16, tag="XZ",
                                name=f"XZ{ci}")[:, :tcc * P]
                XH = gpool.tile([128, TCO * P], fp16, tag="XH",
                                name=f"XH{ci}")[:, :tcc * P]
                gates[ci] = (XR, XZ, XH)
                xk = [xkpool.tile([128, TCO * BC], fp16, tag=f"xk{k}",
                                  name=f"xk{k}_{ci}")[:, :cb]
                      for k in range(KT)]

                def load(k, xk=xk):
                    # 2D view: one contiguous run per partition
                    nc.sync.dma_start(
                        out=xk[k],
                        in_=xT_d.rearrange("(k p) n b -> k p (n b)", p=128)
                        [k, :, co:co + cb])
                loads = [lambda k=k: load(k) for k in range(KT)]

                def scale_bias(g):
                    scale = 0.5 if g == "h" else 1.0
                    if g == "r":
                        bias = -1.0 if fast else brv
                    elif g == "z":
                        bias = 0.0 if fast else bzv
                    else:
                        bias = 0.0
                    return scale, bias

                def mmpair(g, hb, dest, on_dve=False, tcc=tcc, cb=cb, xk=xk,
                           ci=ci):
                    # two hb-halves into one psum tile, ONE wide epilogue
                    ps = pspool.tile([128, 2 * TCO * BC], fp32, tag="mm",
                                     name=f"mm{ci}_{g}{hb}")
                    for half in range(2):
                        for k in range(KT):
                            pe(nc.tensor.matmul(
                                out=ps[:, half * cb:half * cb + cb],
                                lhsT=w_sb[(g, k)][:, (hb + half) * 128:
                                                  (hb + half + 1) * 128],
                                rhs=xk[k],
                                start=(k == 0), stop=(k == KT - 1)))
                    dst = dest.rearrange(
                        "p (t hb b) -> p hb t b", t=tcc, hb=HB)[:, hb:hb + 2]
                    ps4 = ps[:, :2 * cb].rearrange(
                        "p (i t b) -> p i t b", i=2, t=tcc)
                    scale, bias = scale_bias(g)
                    if g == "r" and not fast:
                        bias = brv[:, hb:hb + 1]  # not pair-safe in general
                    if on_dve:   # chunk-0 priming: keep ACT free for s_0
                        sc2 = bias if isinstance(bias, float) else 0.0
                        vop(nc.vector.tensor_scalar(
                            out=dst, in0=ps4, scalar1=scale, scalar2=sc2,
                            op0=OP.mult, op1=OP.add))
                    else:
                        aop(nc.scalar.activation(
                            out=dst, in_=ps4, func=AF.Identity,
                            bias=bias, scale=scale))

                def mmsingle(g, hb, dest, on_dve, tcc=tcc, cb=cb, xk=xk,
                             ci=ci):
                    ps = pspools.tile([128, TCO * BC], fp32, tag="mms",
                                      name=f"mms{ci}_{g}{hb}")
                    for k in range(KT):
                        pe(nc.tensor.matmul(
                            out=ps[:, :cb],
                            lhsT=w_sb[(g, k)][:, hb * 128:(hb + 1) * 128],
                            rhs=xk[k],
                            start=(k == 0), stop=(k == KT - 1)))
                    dst = dest.rearrange(
                        "p (t hb b) -> p t hb b", t=tcc, hb=HB)[:, :, hb, :]
                    ps3 = ps[:, :cb].rearrange("p (t b) -> p t b", t=tcc)
                    scale, bias = scale_bias(g)
                    if g == "r" and not fast:
                        bias = brv[:, hb:hb + 1]
                    elif g == "z" and not fast:
                        bias = bzv[:, hb:hb + 1]
                    if on_dve:
                        sc2 = bias if isinstance(bias, float) else 0.0
                        vop(nc.vector.tensor_scalar(
                            out=dst, in0=ps3, scalar1=scale, scalar2=sc2,
                            op0=OP.mult, op1=OP.add))
                    else:
                        aop(nc.scalar.activation(
                            out=dst, in_=ps3, func=AF.Identity,
                            bias=bias, scale=scale))

                eps_act = []
                eps_dve = []
                if tcc < TCO or not fast:
                    # head / general chunks: all singles; h-gate on DVE in
                    # the fast path to keep ACT's queue short at startup
                    for g, dest in (("r", XR), ("z", XZ), ("h", XH)):
                        for hb in range(HB):
                            dve = fast and g == "h"
                            (eps_dve if dve else eps_act).append(
                                lambda g=g, hb=hb, dest=dest, dve=dve:
                                mmsingle(g, hb, dest, dve))
                else:
                    # steady chunks: 12 pair-epilogues (ACT; chunk-0 priming
                    # overrides z/h onto DVE via on_dve)
                    for g, dest in (("r", XR), ("z", XZ), ("h", XH)):
                        for hb in range(0, HB, 2):
                            eps_act.append(
                                lambda g=g, hb=hb, dest=dest, on_dve=False:
                                mmpair(g, hb, dest, on_dve=on_dve))
                return loads, eps_act, eps_dve

            def emit_output(ci, ring, lo, hi):
                """xbar-transpose steps [lo,hi) of chunk ci's hm ring and
                DMA to ys."""
                nt = hi - lo
                stg = stpool.tile([128, nt * P], fp16, tag="stg",
                                  name=f"stg{ci}_{lo}")
                nc.sync.dma_start_transpose(
                    out=stg.rearrange("p (g m) -> p g m", m=128),
                    in_=ring[:, lo * P:hi * P].rearrange(
                        "p (g u) -> p g u", u=128))
                ot0 = COFFS[ci] - W + lo
                dst = ys_d[:, :, ot0:ot0 + nt, :, :].rearrange(
                    "l b t j c -> (l b) t j c")
                nc.sync.dma_start(
                    out=dst,
                    in_=stg.rearrange("p (t j c) -> p t j c", t=nt, j=JD))

            # ---- emit: prime chunk 0 (x first, then weights, then r-gate
            # epilogues on ACT and z/h on DVE so nothing queues ahead of the
            # first sigmoids) ----
            pend_act = []
            pend_dve = []
            loads0, eps_act0, eps_dve0 = make_pieces(0)
            for p_ in loads0:
                p_()
            for wt, wd in w_dma:
                nc.sync.dma_start(
                    out=wt.rearrange("p (k h) -> p k h", k=KT),
                    in_=wd.rearrange("(k p) h -> p k h", p=128))
            if not fast:
                nc.sync.dma_start(out=brv, in_=brv_d[:, :])
            if fast:
                NP = HB // 2          # pair-groups per gate (4)
                for p_ in eps_act0[:NP]:   # r-pairs -> ACT (feed s_0)
                    p_()
                # z-pairs and half the h-pairs -> DVE; the other h-pairs run
                # on ACT in parallel so the h-gate epilogue tail (the head's
                # binding path after chunk-0's GEMM) halves
                for j, p_ in enumerate(eps_act0[NP:]):
                    if j >= 2 * NP - 2:
                        p_()
                    else:
                        p_(on_dve=True)
            else:
                for p_ in eps_act0:
                    p_()
            for p_ in eps_dve0:
                p_()

            hh, hm = hh0, hm0

            def s_tile(tag, i):
                return spool.tile([128, P], fp16, tag=tag, name=f"{tag}_{i}")

            def gate_col(gt, t):
                return gt[:, t * P:(t + 1) * P]

            # a_0 / c_0
            XR, XZ, XH = gates[0]
            a_t = s_tile("a", 0)
            c_t = s_tile("c", 0)
            if fast:
                vop(nc.vector.tensor_tensor(a_t, gate_col(XR, 0), hh, OP.add))
                vop(nc.vector.tensor_tensor(c_t, gate_col(XZ, 0), hm, OP.add))
            else:
                t1 = s_tile("t1", 0)
                vop(nc.vector.tensor_tensor(t1, mrt, hm, OP.mult))
                vop(nc.vector.tensor_tensor(a_t, t1, gate_col(XR, 0), OP.add))
                t2 = s_tile("t2", 0)
                vop(nc.vector.tensor_tensor(t2, mzt, hm, OP.mult))
                vop(nc.vector.tensor_tensor(c_t, t2, gate_col(XZ, 0), OP.add))

            for ci in range(NCH):
                tcc = CSIZES[ci]
                XR, XZ, XH = gates[ci]
                if ci + 1 < NCH:
                    loads, eps_a, eps_d = make_pieces(ci + 1)
                    for p_ in loads:
                        p_()
                    pend_act.extend(eps_a)
                    pend_dve.extend(eps_d)
                nsteps_left = tcc
                ring = (rpool.tile([128, TCO * P], fp16, tag="ring",
                                   name=f"ring{ci}") if ci >= OC0 else None)
                for t in range(tcc):
                    i = COFFS[ci] + t
                    last = (i == N - 1)
                    na = -(-len(pend_act) // nsteps_left)
                    nd = (-(-len(pend_dve) // nsteps_left)
                          if len(pend_dve) >= nsteps_left else 0)
                    nsteps_left -= 1
                    # chain front: s, sh, e3, q  (a_t from previous tail)
                    s_ = s_tile("s", i)
                    aop(nc.scalar.activation(s_, a_t, AF.Sigmoid, scale=2.0))
                    sh = s_tile("sh", i)
                    vop(nc.vector.tensor_tensor(sh, hm, s_, OP.mult))
                    e3 = s_tile("e3", i)
                    vop(nc.vector.tensor_tensor(e3, sh, gate_col(XH, t),
                                                OP.add))
                    z_ = s_tile("z", i)
                    aop(nc.scalar.activation(z_, c_t, AF.Sigmoid))
                    q_ = s_tile("q", i)
                    aop(nc.scalar.activation(q_, e3, AF.Sigmoid, scale=4.0))
                    # epilogue groups, in ACT's post-q window
                    for _ in range(na):
                        if pend_act:
                            pend_act.pop(0)()
                    for _ in range(nd):
                        if pend_dve:
                            pend_dve.pop(0)()
                    # off-chain tail
                    U2 = s_tile("U2", i)
                    vop(nc.vector.tensor_scalar(
                        out=U2, in0=z_, scalar1=-2.0, scalar2=2.0,
                        op0=OP.mult, op1=OP.add))
                    hz1 = s_tile("hz1", i)
                    vop(nc.vector.tensor_tensor(hz1, hh, z_, OP.mult))
                    v_ = s_tile("vv", i)
                    vop(nc.vector.tensor_tensor(v_, q_, U2, OP.mult))
                    hh_n = s_tile("hh", i)
                    vop(nc.vector.tensor_tensor(hh_n, v_, hz1, OP.add))
                    # chain-critical a' right after hh'
                    if not last and fast:
                        a_t = s_tile("a", i + 1)
                        XRn = gates[ci + 1][0] if t == tcc - 1 else XR
                        vop(nc.vector.tensor_tensor(
                            a_t, hh_n, gate_col(XRn, (t + 1) % tcc
                                                if t == tcc - 1 else t + 1),
                            OP.add))
                    hm_n = (ring[:, t * P:(t + 1) * P] if ring is not None
                            else s_tile("hm", i))
                    vop(nc.vector.tensor_scalar(
                        out=hm_n, in0=hh_n, scalar1=-1.0, scalar2=None,
                        op0=OP.add))
                    if not last:
                        c_t = s_tile("c", i + 1)
                        tn = 0 if t == tcc - 1 else t + 1
                        if fast:
                            XZn = gates[ci + 1][1] if t == tcc - 1 else XZ
                            vop(nc.vector.tensor_tensor(
                                c_t, gate_col(XZn, tn), hm_n, OP.add))
                        else:
                            a_t = s_tile("a", i + 1)
                            XRn = gates[ci + 1][0] if t == tcc - 1 else XR
                            XZn = gates[ci + 1][1] if t == tcc - 1 else XZ
                            t1 = s_tile("t1", i + 1)
                            vop(nc.vector.tensor_tensor(t1, mrt, hm_n,
                                                        OP.mult))
                            vop(nc.vector.tensor_tensor(
                                a_t, t1, gate_col(XRn, tn), OP.add))
                            t2 = s_tile("t2", i + 1)
                            gop(nc.gpsimd.tensor_tensor(t2, mzt, hm_n,
                                                        OP.mult))
                            gop(nc.gpsimd.tensor_tensor(
                                c_t, t2, gate_col(XZn, tn), OP.add))
                    hh = hh_n
                    hm = hm_n
                    # split the last chunk's output to shorten the tail
                    if (ring is not None and ci == NCH - 1
                            and t == tcc // 2 - 1):
                        emit_output(ci, ring, 0, tcc // 2)
                if ring is not None:
                    if ci == NCH - 1:
                        emit_output(ci, ring, tcc // 2, tcc)
                    else:
                        emit_output(ci, ring, 0, tcc)

    _legalize_sync_waits(nc)
    return nc


def _get_nc(fast: bool):
    if fast not in _cache:
        _cache[fast] = _build(fast)
    return _cache[fast]


LAST_RESULT = None


def kernel(**inputs):
    global LAST_RESULT
    from concourse.bass_utils import run_bass_kernel_spmd

    x = np.asarray(inputs["x"], dtype=np.float32)
    kz = np.asarray(inputs["kz"], dtype=np.float32)
    kr = np.asarray(inputs["kr"], dtype=np.float32)
    kh = np.asarray(inputs["kh"], dtype=np.float32)
    mz = np.asarray(inputs["mz"], dtype=np.float32)
    mr = np.asarray(inputs["mr"], dtype=np.float32)
    br = np.asarray(inputs["br"], dtype=np.float32)
    bz = np.asarray(inputs["bz"], dtype=np.float32)
    assert x.shape == (B, T, D) and kz.shape == (D, H)

    fast = bool(np.all(mz == 1.0) and np.all(mr == 1.0)
                and np.all(br == 0.0) and np.all(bz == 0.0)
                and not int(os.environ.get("FORCE_SLOW", "0")))
    nc = _get_nc(fast)

    def pvec(v):  # [H] -> [128, HB] with [h_a, h_b]
        return np.ascontiguousarray(v.reshape(HB, 128).T)

    def ptile(v):  # [H] -> [128, (hb, b)] fp16, replicated over b
        t = v.reshape(HB, 128).T
        return np.ascontiguousarray(
            np.repeat(t[:, :, None], BC, axis=2).reshape(128, P)
        ).astype(np.float16)

    base = {
        "kz": np.ascontiguousarray(kz).astype(np.float16),
        "kr": np.ascontiguousarray(kr).astype(np.float16),
        "kh": np.ascontiguousarray(kh).astype(np.float16),
        "brv": pvec((br - mr) if fast else br).astype(np.float32),
    }
    if not fast:
        base["bzv"] = pvec(bz).astype(np.float32)
        base["mrt"] = ptile(mr)
        base["mzt"] = ptile(mz)

    x16 = x.astype(np.float16)
    in_maps = []
    for i in range(NCORES):
        i_t, i_b = i // SB, i % SB
        t0 = i_t * SEG
        bs = slice(i_b * BC, (i_b + 1) * BC)
        xc = np.zeros((BC, N, D), np.float16)
        src = x16[bs, max(0, t0 - W):t0 + SEG]
        xc[:, N - src.shape[1]:, :] = src
        xTc = np.ascontiguousarray(xc.transpose(2, 1, 0))
        in_maps.append(dict(base, xT=xTc))

    trace = bool(int(os.environ.get("KERNEL_TRACE", "0")))
    res = run_bass_kernel_spmd(nc, in_maps, list(range(NCORES)), trace=trace)
    LAST_RESULT = res
    ys = np.empty((B, T, H), np.float32)
    for i in range(NCORES):
        i_t, i_b = i // SB, i % SB
        yc = res.results[i]["ys"].astype(np.float32)  # [l, b, t, j, c]
        ys[i_b * BC:(i_b + 1) * BC, i_t * SEG:(i_t + 1) * SEG, :] = (
            yc.transpose(1, 2, 3, 0, 4).reshape(BC, SEG, H))
    return ys


# revision 43
# speedup vs baseline: 1.0038x; 1.0002x over previous
"""Trainium2 Bass kernel for nn_BRC_17179869451 (BRC-style RNN).

  xz/xr/xh = x @ {kz,kr,kh}   (three [B*T,D]x[D,H] GEMMs)
  scan over T:
      r = tanh(xr_t + h*mr + br) + 1
      z = sigmoid(xz_t + h*mz + bz)
      h = z*h + (1-z)*tanh(xh_t + r*h)

Sharding (8 cores = 8 time-segments, all 64 batches per core): the BRC
forget gate makes h_t depend only weakly on the distant past, so each
core computes a 64-step time segment for all 64 batches, preceded by a
W=24-step redundant warmup from h=0.  Segment 0 zero-pads its warmup
input, which keeps h exactly 0.

Everything on-device runs fp16: fp16 GEMMs, fp16 scan ops (DVE 2x perf
mode), fp16 output staged via the xbar DMA-transpose and upcast to
fp32 on the host.  Wide [128,512] ops amortize per-instruction
overhead.  Chunks: two 4-step head chunks (short time-to-first-step),
then 8-step chunks.  GEMM epilogues (PSUM->SBUF cast+affine) mostly run
as [128,1024] hb-pairs on ACT in each step's post-sigmoid window; the
last h-gate pair runs as two singles on DVE late in the chunk so the
next chunk's first steps never wait on ACT's epilogue tail.

Per-step math (fast path mz=mr=1; hh = h+1 shifted state, hm = h):
  s = sigmoid(2*(xr-1 + hh))            r = 2s
  q = sigmoid(4*(hm*s + xh/2))          tanh(xh + r*h) = 2q-1
  z = sigmoid(xz + hm)
  hh' = 2q(1-z) + hh*z ;  ys = hm' = hh' - 1
Layout per core: state [128 x 512]: partition h_a = h mod 128, free
(hb = h div 128 [8], b [64]).  Output: per 8-step chunk the hm ring
[128, (t,j,u)] is xbar-transposed to [u, (t,j), h_a] and DMA'd to
ys[l,b,t,j,c]; host reassembles to [b, t, h].
"""

import os
import numpy as np

B, T, D, H = 64, 512, 512, 1024
NCORES = 8
ST = 8                    # time segments
SB = 1                    # batch shards
BC = B // SB              # 64 batches per core
SEG = T // ST             # 64 output steps per core
W = 24                    # warmup steps
N = SEG + W               # 88 steps computed per core
CSIZES = [8] * 11                    # per-chunk step counts
COFFS = [sum(CSIZES[:i]) for i in range(len(CSIZES))]
NCH = len(CSIZES)         # 11 chunks
OC0 = 3                   # first output chunk (step 24)
TCO = 8                   # steps per output chunk
HB = H // 128             # 8 h-blocks
P = HB * BC               # 512 = free size of scan state
KT = D // 128             # 4 k-tiles
LJ = 128 // BC            # h-blocks packed per 128-partition u-group (2)
JD = P // 128             # u-groups per step; j-dim of ys (4)

_cache = {}


def _apply_tile_drain_patch():
    """Spread end-of-kernel sem waits over single-wait sync nops: walrus
    CoreV3 codegen rejects the stock Tile exit Drain that carries one wait
    per logical proc ("Too many sync wait commands")."""
    import concourse.tile as tile_mod

    if getattr(tile_mod.TileContext, "_drain_patched", False):
        return

    def _patched(self, tick_clock, wait_clock):
        from concourse.vector_clock import ScopedClock

        vclock = tick_clock.global_clock
        pend = [(p, vclock[p]) for p in range(len(vclock)) if vclock[p] > 0]
        for proc, tick in pend:
            sub = ScopedClock()
            sub.require_at_least(None, proc, tick)
            nop_inst = self.nc.sync.nop(nofuse=True)
            wait_clock.add_sem_waits(nop_inst.ins, sub)
        self.nc.sync.drain()
        self.nc.all_engine_barrier()
        assert self.sems is not None
        popped = self.nc._tile_sem_poison_stack.pop()
        assert popped is self._sem_poison
        self.nc.clear_and_free_semaphores(list(self.sems.allocated().values()))
        self.nc.all_engine_barrier()

    tile_mod.TileContext._drain_and_barrier = _patched
    tile_mod.TileContext._drain_patched = True


def _legalize_sync_waits(nc, max_waits: int = 1):
    """walrus codegen here rejects instructions with >1 sem wait ("Too many
    sync wait commands"); hoist extra waits onto same-engine NoOps."""
    import concourse.mybir as mybir

    n = 0
    for f in nc.m.functions:
        for bb in f.blocks:
            out = []
            for ins in bb.instructions:
                si = ins.sync_info
                if si is not None and si.on_wait and len(si.on_wait) > max_waits:
                    waits = list(si.on_wait)
                    for w in waits[:-max_waits]:
                        n += 1
                        nop = mybir.InstNoOp(
                            name=f"waitnop_{n}", engine=ins.engine)
                        nop.sync_info = mybir.SyncInfo(
                            on_wait=[w], on_update=[])
                        out.append(nop)
                    si.on_wait = waits[-max_waits:]
                out.append(ins)
            bb.instructions = out


def _build(fast: bool):
    import concourse.bass as bass
    import concourse.mybir as mybir
    from concourse.tile import TileContext

    _apply_tile_drain_patch()

    fp16 = mybir.dt.float16
    fp32 = mybir.dt.float32
    AF = mybir.ActivationFunctionType
    OP = mybir.AluOpType

    nc = bass.Bass()
    # const AP for the fast-path XR epilogue bias (br - mr = -1)
    _cb = nc.alloc_sbuf_tensor("const-f32-neg1", [128, 1], fp32)
    nc.gpsimd.memset(_cb.ap(), -1.0)
    nc.const_aps.aps[(fp32, -1.0)] = _cb.ap()
    nc.all_engine_barrier()
    xT_d = nc.dram_tensor("xT", [D, N, BC], fp16, kind="ExternalInput")
    kz_d = nc.dram_tensor("kz", [D, H], fp16, kind="ExternalInput")
    kr_d = nc.dram_tensor("kr", [D, H], fp16, kind="ExternalInput")
    kh_d = nc.dram_tensor("kh", [D, H], fp16, kind="ExternalInput")
    brv_d = nc.dram_tensor("brv", [128, HB], fp32, kind="ExternalInput")
    if not fast:
        bzv_d = nc.dram_tensor("bzv", [128, HB], fp32, kind="ExternalInput")
        mrt_d = nc.dram_tensor("mrt", [128, P], fp16, kind="ExternalInput")
        mzt_d = nc.dram_tensor("mzt", [128, P], fp16, kind="ExternalInput")
    # ys stored [l, b, t, j, c] (h = (j*LJ+l)*128+c) so the post-transpose
    # chunk DMA is perfectly linear; host reassembles to [b, t, h].
    ys_d = nc.dram_tensor("ys", [LJ, BC, SEG, JD, 128], fp16,
                          kind="ExternalOutput")

    with TileContext(nc) as tc:
        with (
            tc.tile_pool(name="const", bufs=1) as cpool,
            tc.tile_pool(name="xk", bufs=2) as xkpool,
            tc.tile_pool(name="gates", bufs=3) as gpool,
            tc.tile_pool(name="scan", bufs=3) as spool,
            tc.tile_pool(name="ring", bufs=2) as rpool,
            tc.tile_pool(name="stg", bufs=2) as stpool,
            tc.tile_pool(name="psmm", bufs=3, space="PSUM") as pspool,
            tc.tile_pool(name="psms", bufs=2, space="PSUM") as pspools,
        ):
            # ---- weight / bias tiles (DMAs emitted after chunk-0 x) ----
            w_sb = {}
            w_dma = []
            for name, wd in (("r", kr_d), ("h", kh_d), ("z", kz_d)):
                wt = cpool.tile([128, KT * H], fp16, tag=f"w{name}",
                                name=f"w{name}")
                w_dma.append((wt, wd))
                for k in range(KT):
                    w_sb[(name, k)] = wt[:, k * H:(k + 1) * H]
            brv = cpool.tile([128, HB], fp32, tag="brv", name="brv")
            if not fast:
                bzv = cpool.tile([128, HB], fp32, tag="bzv", name="bzv")
                nc.sync.dma_start(out=bzv, in_=bzv_d[:, :])
                mrt = cpool.tile([128, P], fp16, tag="mrt", name="mrt")
                nc.sync.dma_start(out=mrt, in_=mrt_d[:, :])
                mzt = cpool.tile([128, P], fp16, tag="mzt", name="mzt")
                nc.sync.dma_start(out=mzt, in_=mzt_d[:, :])

            hh0 = cpool.tile([128, P], fp16, tag="hh0", name="hh0")
            nc.vector.memset(hh0, 1.0)   # hh = h+1, h0 = 0
            hm0 = cpool.tile([128, P], fp16, tag="hm0", name="hm0")
            nc.vector.memset(hm0, 0.0)

            import bass_rust as _br

            _last = {}

            def _pin(eng, bi):
                # Pin each engine's stream to emission order; prevents
                # scheduler priority inversions (engines execute in-order).
                if eng in _last:
                    _br.add_dep_helper(bi.ins, _last[eng].ins, sync=False,
                                       reason=f"{eng} emission order")
                _last[eng] = bi
                return bi

            def vop(bi):
                return _pin("v", bi)

            def aop(bi):
                return _pin("a", bi)

            def gop(bi):
                return _pin("g", bi)

            def pe(bi):
                return _pin("pe", bi)

            # ---- GEMM pieces per chunk ----
            gates = {}   # ci -> (XR, XZ, XH) sbuf tiles [128, tc*P] fp16

            def make_pieces(ci):
                """Returns (loads, eps_act, eps_dve): closures for chunk ci's
                x load and (gate,hb) matmul+epilogue groups split by the
                engine that runs the epilogue."""
                tcc = CSIZES[ci]
                cb = tcc * BC
                co = COFFS[ci] * BC
                # pool tags need constant shapes: allocate steady-size,
                # slice for the short head chunks
                XR = gpool.tile([128, TCO * P], fp16, tag="XR",
                                name=f"XR{ci}")[:, :tcc * P]
                XZ = gpool.tile([128, TCO * P], fp16, tag="XZ",
                                name=f"XZ{ci}")[:, :tcc * P]
                XH = gpool.tile([128, TCO * P], fp16, tag="XH",
                                name=f"XH{ci}")[:, :tcc * P]
                gates[ci] = (XR, XZ, XH)
                xk = [xkpool.tile([128, TCO * BC], fp16, tag=f"xk{k}",
                                  name=f"xk{k}_{ci}")[:, :cb]
                      for k in range(KT)]

                def load(k, xk=xk):
                    # 2D view: one contiguous run per partition
                    nc.sync.dma_start(
                        out=xk[k],
                        in_=xT_d.rearrange("(k p) n b -> k p (n b)", p=128)
                        [k, :, co:co + cb])
                loads = [lambda k=k: load(k) for k in range(KT)]

                def scale_bias(g):
                    scale = 0.5 if g == "h" else 1.0
                    if g == "r":
                        bias = -1.0 if fast else brv
                    elif g == "z":
                        bias = 0.0 if fast else bzv
                    else:
                        bias = 0.0
                    return scale, bias

                def mmpair(g, hb, dest, on_dve=False, tcc=tcc, cb=cb, xk=xk,
                           ci=ci):
                    # two hb-halves into one psum tile, ONE wide epilogue
                    ps = pspool.tile([128, 2 * TCO * BC], fp32, tag="mm",
                                     name=f"mm{ci}_{g}{hb}")
                    for half in range(2):
                        for k in range(KT):
                            pe(nc.tensor.matmul(
                                out=ps[:, half * cb:half * cb + cb],
                                lhsT=w_sb[(g, k)][:, (hb + half) * 128:
                                                  (hb + half + 1) * 128],
                                rhs=xk[k],
                                start=(k == 0), stop=(k == KT - 1)))
                    dst = dest.rearrange(
                        "p (t hb b) -> p hb t b", t=tcc, hb=HB)[:, hb:hb + 2]
                    ps4 = ps[:, :2 * cb].rearrange(
                        "p (i t b) -> p i t b", i=2, t=tcc)
                    scale, bias = scale_bias(g)
                    if g == "r" and not fast:
                        bias = brv[:, hb:hb + 1]  # not pair-safe in general
                    if on_dve:   # chunk-0 priming: keep ACT free for s_0
                        sc2 = bias if isinstance(bias, float) else 0.0
                        vop(nc.vector.tensor_scalar(
                            out=dst, in0=ps4, scalar1=scale, scalar2=sc2,
                            op0=OP.mult, op1=OP.add))
                    else:
                        aop(nc.scalar.activation(
                            out=dst, in_=ps4, func=AF.Identity,
                            bias=bias, scale=scale))

                def mmsingle(g, hb, dest, on_dve, tcc=tcc, cb=cb, xk=xk,
                             ci=ci):
                    ps = pspools.tile([128, TCO * BC], fp32, tag="mms",
                                      name=f"mms{ci}_{g}{hb}")
                    for k in range(KT):
                        pe(nc.tensor.matmul(
                            out=ps[:, :cb],
                            lhsT=w_sb[(g, k)][:, hb * 128:(hb + 1) * 128],
                            rhs=xk[k],
                            start=(k == 0), stop=(k == KT - 1)))
                    dst = dest.rearrange(
                        "p (t hb b) -> p t hb b", t=tcc, hb=HB)[:, :, hb, :]
                    ps3 = ps[:, :cb].rearrange("p (t b) -> p t b", t=tcc)
                    scale, bias = scale_bias(g)
                    if g == "r" and not fast:
                        bias = brv[:, hb:hb + 1]
                    elif g == "z" and not fast:
                        bias = bzv[:, hb:hb + 1]
                    if on_dve:
                        sc2 = bias if isinstance(bias, float) else 0.0
                        vop(nc.vector.tensor_scalar(
                            out=dst, in0=ps3, scalar1=scale, scalar2=sc2,
                            op0=OP.mult, op1=OP.add))
                    else:
                        aop(nc.scalar.activation(
                            out=dst, in_=ps3, func=AF.Identity,
                            bias=bias, scale=scale))

                eps_act = []
                eps_dve = []
                if tcc < TCO or not fast:
                    # head / general chunks: all singles; h-gate on DVE in
                    # the fast path to keep ACT's queue short at startup
                    for g, dest in (("r", XR), ("z", XZ), ("h", XH)):
                        for hb in range(HB):
                            dve = fast and g == "h"
                            (eps_dve if dve else eps_act).append(
                                lambda g=g, hb=hb, dest=dest, dve=dve:
                                mmsingle(g, hb, dest, dve))
                else:
                    # steady chunks: 12 pair-epilogues (ACT; chunk-0 priming
                    # overrides z/h onto DVE via on_dve)
                    for g, dest in (("r", XR), ("z", XZ), ("h", XH)):
                        for hb in range(0, HB, 2):
                            eps_act.append(
                                lambda g=g, hb=hb, dest=dest, on_dve=False:
                                mmpair(g, hb, dest, on_dve=on_dve))
                return loads, eps_act, eps_dve

            def emit_output(ci, ring, lo, hi):
                """xbar-transpose steps [lo,hi) of chunk ci's hm ring and
                DMA to ys."""
                nt = hi - lo
                stg = stpool.tile([128, nt * P], fp16, tag="stg",
                                  name=f"stg{ci}_{lo}")
                nc.sync.dma_start_transpose(
                    out=stg.rearrange("p (g m) -> p g m", m=128),
                    in_=ring[:, lo * P:hi * P].rearrange(
                        "p (g u) -> p g u", u=128))
                ot0 = COFFS[ci] - W + lo
                dst = ys_d[:, :, ot0:ot0 + nt, :, :].rearrange(
                    "l b t j c -> (l b) t j c")
                nc.sync.dma_start(
                    out=dst,
                    in_=stg.rearrange("p (t j c) -> p t j c", t=nt, j=JD))

            # ---- emit: prime chunk 0 (x first, then weights, then r-gate
            # epilogues on ACT and z/h on DVE so nothing queues ahead of the
            # first sigmoids) ----
            pend_act = []
            pend_dve = []
            loads0, eps_act0, eps_dve0 = make_pieces(0)
            for p_ in loads0:
                p_()
            for wt, wd in w_dma:
                nc.sync.dma_start(
                    out=wt.rearrange("p (k h) -> p k h", k=KT),
                    in_=wd.rearrange("(k p) h -> p k h", p=128))
            if not fast:
                nc.sync.dma_start(out=brv, in_=brv_d[:, :])
            if fast:
                NP = HB // 2          # pair-groups per gate (4)
                for p_ in eps_act0[:NP]:   # r-pairs -> ACT (feed s_0)
                    p_()
                # z-pairs and half the h-pairs -> DVE; the other h-pairs run
                # on ACT in parallel so the h-gate epilogue tail (the head's
                # binding path after chunk-0's GEMM) halves
                for j, p_ in enumerate(eps_act0[NP:]):
                    if j >= 2 * NP - 2:
                        p_()
                    else:
                        p_(on_dve=True)
            else:
                for p_ in eps_act0:
                    p_()
            for p_ in eps_dve0:
                p_()

            hh, hm = hh0, hm0

            def s_tile(tag, i):
                return spool.tile([128, P], fp16, tag=tag, name=f"{tag}_{i}")

            def gate_col(gt, t):
                return gt[:, t * P:(t + 1) * P]

            # a_0 / c_0
            XR, XZ, XH = gates[0]
            a_t = s_tile("a", 0)
            c_t = s_tile("c", 0)
            if fast:
                vop(nc.vector.tensor_tensor(a_t, gate_col(XR, 0), hh, OP.add))
                vop(nc.vector.tensor_tensor(c_t, gate_col(XZ, 0), hm, OP.add))
            else:
                t1 = s_tile("t1", 0)
                vop(nc.vector.tensor_tensor(t1, mrt, hm, OP.mult))
                vop(nc.vector.tensor_tensor(a_t, t1, gate_col(XR, 0), OP.add))
                t2 = s_tile("t2", 0)
                vop(nc.vector.tensor_tensor(t2, mzt, hm, OP.mult))
                vop(nc.vector.tensor_tensor(c_t, t2, gate_col(XZ, 0), OP.add))

            for ci in range(NCH):
                tcc = CSIZES[ci]
                XR, XZ, XH = gates[ci]
                if ci + 1 < NCH:
                    loads, eps_a, eps_d = make_pieces(ci + 1)
                    for p_ in loads:
                        p_()
                    pend_act.extend(eps_a)
                    pend_dve.extend(eps_d)
                nsteps_left = tcc
                ring = (rpool.tile([128, TCO * P], fp16, tag="ring",
                                   name=f"ring{ci}") if ci >= OC0 else None)
                for t in range(tcc):
                    i = COFFS[ci] + t
                    last = (i == N - 1)
                    if fast and tcc == TCO and nsteps_left > 1:
                        # alternating 1/2 keeps ACT's epilogue backlog (and
                        # so the sigmoid queue delay) minimal vs ceil-spread
                        na = min(len(pend_act), 1 if t % 2 == 0 else 2)
                    else:
                        na = -(-len(pend_act) // max(nsteps_left, 1))
                    nd = (-(-len(pend_dve) // nsteps_left)
                          if len(pend_dve) >= nsteps_left else 0)
                    nsteps_left -= 1
                    # chain front: s, sh, e3, q  (a_t from previous tail)
                    s_ = s_tile("s", i)
                    aop(nc.scalar.activation(s_, a_t, AF.Sigmoid, scale=2.0))
                    sh = s_tile("sh", i)
                    vop(nc.vector.tensor_tensor(sh, hm, s_, OP.mult))
                    e3 = s_tile("e3", i)
                    vop(nc.vector.tensor_tensor(e3, sh, gate_col(XH, t),
                                                OP.add))
                    z_ = s_tile("z", i)
                    aop(nc.scalar.activation(z_, c_t, AF.Sigmoid))
                    q_ = s_tile("q", i)
                    aop(nc.scalar.activation(q_, e3, AF.Sigmoid, scale=4.0))
                    # epilogue groups, in ACT's post-q window
                    for _ in range(na):
                        if pend_act:
                            pend_act.pop(0)()
                    for _ in range(nd):
                        if pend_dve:
                            pend_dve.pop(0)()
                    # off-chain tail
                    U2 = s_tile("U2", i)
                    vop(nc.vector.tensor_scalar(
                        out=U2, in0=z_, scalar1=-2.0, scalar2=2.0,
                        op0=OP.mult, op1=OP.add))
                    hz1 = s_tile("hz1", i)
                    vop(nc.vector.tensor_tensor(hz1, hh, z_, OP.mult))
                    v_ = s_tile("vv", i)
                    vop(nc.vector.tensor_tensor(v_, q_, U2, OP.mult))
                    hh_n = s_tile("hh", i)
                    vop(nc.vector.tensor_tensor(hh_n, v_, hz1, OP.add))
                    # chain-critical a' right after hh'
                    if not last and fast:
                        a_t = s_tile("a", i + 1)
                        XRn = gates[ci + 1][0] if t == tcc - 1 else XR
                        vop(nc.vector.tensor_tensor(
                            a_t, hh_n, gate_col(XRn, (t + 1) % tcc
                                                if t == tcc - 1 else t + 1),
                            OP.add))
                    hm_n = (ring[:, t * P:(t + 1) * P] if ring is not None
                            else s_tile("hm", i))
                    vop(nc.vector.tensor_scalar(
                        out=hm_n, in0=hh_n, scalar1=-1.0, scalar2=None,
                        op0=OP.add))
                    if not last:
                        c_t = s_tile("c", i + 1)
                        tn = 0 if t == tcc - 1 else t + 1
                        if fast:
                            XZn = gates[ci + 1][1] if t == tcc - 1 else XZ
                            vop(nc.vector.tensor_tensor(
                                c_t, gate_col(XZn, tn), hm_n, OP.add))
                        else:
                            a_t = s_tile("a", i + 1)
                            XRn = gates[ci + 1][0] if t == tcc - 1 else XR
                            XZn = gates[ci + 1][1] if t == tcc - 1 else XZ
                            t1 = s_tile("t1", i + 1)
                            vop(nc.vector.tensor_tensor(t1, mrt, hm_n,
                                                        OP.mult))
                            vop(nc.vector.tensor_tensor(
                                a_t, t1, gate_col(XRn, tn), OP.add))
                            t2 = s_tile("t2", i + 1)
                            gop(nc.gpsimd.tensor_tensor(t2, mzt, hm_n,
                                                        OP.mult))
                            gop(nc.gpsimd.tensor_tensor(
                                c_t, t2, gate_col(XZn, tn), OP.add))
                    hh = hh_n
                    hm = hm_n
                    # split the last chunk's output to shorten the tail
                    if (ring is not None and ci == NCH - 1
                            and t == tcc // 2 - 1):
                        emit_output(ci, ring, 0, tcc // 2)
                if ring is not None:
                    if ci == NCH - 1:
                        emit_output(ci, ring, tcc // 2, tcc)
                    else:
                        emit_output(ci, ring, 0, tcc)

    _legalize_sync_waits(nc)
    return nc


def _get_nc(fast: bool):
    if fast not in _cache:
        _cache[fast] = _build(fast)
    return _cache[fast]


LAST_RESULT = None


def kernel(**inputs):
    global LAST_RESULT
    from concourse.bass_utils import run_bass_kernel_spmd

    x = np.asarray(inputs["x"], dtype=np.float32)
    kz = np.asarray(inputs["kz"], dtype=np.float32)
    kr = np.asarray(inputs["kr"], dtype=np.float32)
    kh = np.asarray(inputs["kh"], dtype=np.float32)
    mz = np.asarray(inputs["mz"], dtype=np.float32)
    mr = np.asarray(inputs["mr"], dtype=np.float32)
    br = np.asarray(inputs["br"], dtype=np.float32)
    bz = np.asarray(inputs["bz"], dtype=np.float32)
    assert x.shape == (B, T, D) and kz.shape == (D, H)

    fast = bool(np.all(mz == 1.0) and np.all(mr == 1.0)
                and np.all(br == 0.0) and np.all(bz == 0.0)
                and not int(os.environ.get("FORCE_SLOW", "0")))
    nc = _get_nc(fast)

    def pvec(v):  # [H] -> [128, HB] with [h_a, h_b]
        return np.ascontiguousarray(v.reshape(HB, 128).T)

    def ptile(v):  # [H] -> [128, (hb, b)] fp16, replicated over b
        t = v.reshape(HB, 128).T
        return np.ascontiguousarray(
            np.repeat(t[:, :, None], BC, axis=2).reshape(128, P)
        ).astype(np.float16)

    base = {
        "kz": np.ascontiguousarray(kz).astype(np.float16),
        "kr": np.ascontiguousarray(kr).astype(np.float16),
        "kh": np.ascontiguousarray(kh).astype(np.float16),
        "brv": pvec((br - mr) if fast else br).astype(np.float32),
    }
    if not fast:
        base["bzv"] = pvec(bz).astype(np.float32)
        base["mrt"] = ptile(mr)
        base["mzt"] = ptile(mz)

    x16 = x.astype(np.float16)
    in_maps = []
    for i in range(NCORES):
        i_t, i_b = i // SB, i % SB
        t0 = i_t * SEG
        bs = slice(i_b * BC, (i_b + 1) * BC)
        xc = np.zeros((BC, N, D), np.float16)
        src = x16[bs, max(0, t0 - W):t0 + SEG]
        xc[:, N - src.shape[1]:, :] = src
        xTc = np.ascontiguousarray(xc.transpose(2, 1, 0))
        in_maps.append(dict(base, xT=xTc))

    trace = bool(int(os.environ.get("KERNEL_TRACE", "0")))
    res = run_bass_kernel_spmd(nc, in_maps, list(range(NCORES)), trace=trace)
    LAST_RESULT = res
    ys = np.empty((B, T, H), np.float32)
    for i in range(NCORES):
        i_t, i_b = i // SB, i % SB
        yc = res.results[i]["ys"].astype(np.float32)  # [l, b, t, j, c]
        ys[i_b * BC:(i_b + 1) * BC, i_t * SEG:(i_t + 1) * SEG, :] = (
            yc.transpose(1, 2, 3, 0, 4).reshape(BC, SEG, H))
    return ys


# revision 44
# speedup vs baseline: 1.0099x; 1.0061x over previous
"""Trainium2 Bass kernel for nn_BRC_17179869451 (BRC-style RNN).

  xz/xr/xh = x @ {kz,kr,kh}   (three [B*T,D]x[D,H] GEMMs)
  scan over T:
      r = tanh(xr_t + h*mr + br) + 1
      z = sigmoid(xz_t + h*mz + bz)
      h = z*h + (1-z)*tanh(xh_t + r*h)

Sharding (8 cores = 8 time-segments, all 64 batches per core): the BRC
forget gate makes h_t depend only weakly on the distant past, so each
core computes a 64-step time segment for all 64 batches, preceded by a
W=24-step redundant warmup from h=0.  Segment 0 zero-pads its warmup
input, which keeps h exactly 0.

Everything on-device runs fp16: fp16 GEMMs, fp16 scan ops (DVE 2x perf
mode), fp16 output staged via the xbar DMA-transpose and upcast to
fp32 on the host.  Wide [128,512] ops amortize per-instruction
overhead.  Chunks: two 4-step head chunks (short time-to-first-step),
then 8-step chunks.  GEMM epilogues (PSUM->SBUF cast+affine) mostly run
as [128,1024] hb-pairs on ACT in each step's post-sigmoid window; the
last h-gate pair runs as two singles on DVE late in the chunk so the
next chunk's first steps never wait on ACT's epilogue tail.

Per-step math (fast path mz=mr=1; hh = h+1 shifted state, hm = h):
  s = sigmoid(2*(xr-1 + hh))            r = 2s
  q = sigmoid(4*(hm*s + xh/2))          tanh(xh + r*h) = 2q-1
  z = sigmoid(xz + hm)
  hh' = 2q(1-z) + hh*z ;  ys = hm' = hh' - 1
Layout per core: state [128 x 512]: partition h_a = h mod 128, free
(hb = h div 128 [8], b [64]).  Output: per 8-step chunk the hm ring
[128, (t,j,u)] is xbar-transposed to [u, (t,j), h_a] and DMA'd to
ys[l,b,t,j,c]; host reassembles to [b, t, h].
"""

import os
import numpy as np

B, T, D, H = 64, 512, 512, 1024
NCORES = 8
ST = 8                    # time segments
SB = 1                    # batch shards
BC = B // SB              # 64 batches per core
SEG = T // ST             # 64 output steps per core
W = 24                    # warmup steps
N = SEG + W               # 88 steps computed per core
CSIZES = [8] * 11                    # per-chunk step counts
COFFS = [sum(CSIZES[:i]) for i in range(len(CSIZES))]
NCH = len(CSIZES)         # 11 chunks
OC0 = 3                   # first output chunk (step 24)
TCO = 8                   # steps per output chunk
HB = H // 128             # 8 h-blocks
P = HB * BC               # 512 = free size of scan state
KT = D // 128             # 4 k-tiles
LJ = 128 // BC            # h-blocks packed per 128-partition u-group (2)
JD = P // 128             # u-groups per step; j-dim of ys (4)

_cache = {}


def _apply_tile_drain_patch():
    """Spread end-of-kernel sem waits over single-wait sync nops: walrus
    CoreV3 codegen rejects the stock Tile exit Drain that carries one wait
    per logical proc ("Too many sync wait commands")."""
    import concourse.tile as tile_mod

    if getattr(tile_mod.TileContext, "_drain_patched", False):
        return

    def _patched(self, tick_clock, wait_clock):
        from concourse.vector_clock import ScopedClock

        vclock = tick_clock.global_clock
        pend = [(p, vclock[p]) for p in range(len(vclock)) if vclock[p] > 0]
        for proc, tick in pend:
            sub = ScopedClock()
            sub.require_at_least(None, proc, tick)
            nop_inst = self.nc.sync.nop(nofuse=True)
            wait_clock.add_sem_waits(nop_inst.ins, sub)
        self.nc.sync.drain()
        self.nc.all_engine_barrier()
        assert self.sems is not None
        popped = self.nc._tile_sem_poison_stack.pop()
        assert popped is self._sem_poison
        self.nc.clear_and_free_semaphores(list(self.sems.allocated().values()))
        self.nc.all_engine_barrier()

    tile_mod.TileContext._drain_and_barrier = _patched
    tile_mod.TileContext._drain_patched = True


def _legalize_sync_waits(nc, max_waits: int = 1):
    """walrus codegen here rejects instructions with >1 sem wait ("Too many
    sync wait commands"); hoist extra waits onto same-engine NoOps."""
    import concourse.mybir as mybir

    n = 0
    for f in nc.m.functions:
        for bb in f.blocks:
            out = []
            for ins in bb.instructions:
                si = ins.sync_info
                if si is not None and si.on_wait and len(si.on_wait) > max_waits:
                    waits = list(si.on_wait)
                    for w in waits[:-max_waits]:
                        n += 1
                        nop = mybir.InstNoOp(
                            name=f"waitnop_{n}", engine=ins.engine)
                        nop.sync_info = mybir.SyncInfo(
                            on_wait=[w], on_update=[])
                        out.append(nop)
                    si.on_wait = waits[-max_waits:]
                out.append(ins)
            bb.instructions = out


def _build(fast: bool):
    import concourse.bass as bass
    import concourse.mybir as mybir
    from concourse.tile import TileContext

    _apply_tile_drain_patch()

    fp16 = mybir.dt.float16
    fp32 = mybir.dt.float32
    AF = mybir.ActivationFunctionType
    OP = mybir.AluOpType

    nc = bass.Bass()
    # const AP for the fast-path XR epilogue bias (br - mr = -1)
    _cb = nc.alloc_sbuf_tensor("const-f32-neg1", [128, 1], fp32)
    nc.gpsimd.memset(_cb.ap(), -1.0)
    nc.const_aps.aps[(fp32, -1.0)] = _cb.ap()
    nc.all_engine_barrier()
    xT_d = nc.dram_tensor("xT", [D, N, BC], fp16, kind="ExternalInput")
    kz_d = nc.dram_tensor("kz", [D, H], fp16, kind="ExternalInput")
    kr_d = nc.dram_tensor("kr", [D, H], fp16, kind="ExternalInput")
    kh_d = nc.dram_tensor("kh", [D, H], fp16, kind="ExternalInput")
    brv_d = nc.dram_tensor("brv", [128, HB], fp32, kind="ExternalInput")
    if not fast:
        bzv_d = nc.dram_tensor("bzv", [128, HB], fp32, kind="ExternalInput")
        mrt_d = nc.dram_tensor("mrt", [128, P], fp16, kind="ExternalInput")
        mzt_d = nc.dram_tensor("mzt", [128, P], fp16, kind="ExternalInput")
    # ys stored [l, b, t, j, c] (h = (j*LJ+l)*128+c) so the post-transpose
    # chunk DMA is perfectly linear; host reassembles to [b, t, h].
    ys_d = nc.dram_tensor("ys", [LJ, BC, SEG, JD, 128], fp16,
                          kind="ExternalOutput")

    with TileContext(nc) as tc:
        with (
            tc.tile_pool(name="const", bufs=1) as cpool,
            tc.tile_pool(name="xk", bufs=2) as xkpool,
            tc.tile_pool(name="gates", bufs=3) as gpool,
            tc.tile_pool(name="scan", bufs=3) as spool,
            tc.tile_pool(name="ring", bufs=2) as rpool,
            tc.tile_pool(name="stg", bufs=2) as stpool,
            tc.tile_pool(name="psmm", bufs=3, space="PSUM") as pspool,
            tc.tile_pool(name="psms", bufs=2, space="PSUM") as pspools,
        ):
            # ---- weight / bias tiles (DMAs emitted after chunk-0 x) ----
            w_sb = {}
            w_dma = []
            for name, wd in (("r", kr_d), ("h", kh_d), ("z", kz_d)):
                wt = cpool.tile([128, KT * H], fp16, tag=f"w{name}",
                                name=f"w{name}")
                w_dma.append((wt, wd))
                for k in range(KT):
                    w_sb[(name, k)] = wt[:, k * H:(k + 1) * H]
            brv = cpool.tile([128, HB], fp32, tag="brv", name="brv")
            if not fast:
                bzv = cpool.tile([128, HB], fp32, tag="bzv", name="bzv")
                nc.sync.dma_start(out=bzv, in_=bzv_d[:, :])
                mrt = cpool.tile([128, P], fp16, tag="mrt", name="mrt")
                nc.sync.dma_start(out=mrt, in_=mrt_d[:, :])
                mzt = cpool.tile([128, P], fp16, tag="mzt", name="mzt")
                nc.sync.dma_start(out=mzt, in_=mzt_d[:, :])

            hh0 = cpool.tile([128, P], fp16, tag="hh0", name="hh0")
            nc.vector.memset(hh0, 1.0)   # hh = h+1, h0 = 0
            hm0 = cpool.tile([128, P], fp16, tag="hm0", name="hm0")
            nc.vector.memset(hm0, 0.0)

            import bass_rust as _br

            _last = {}

            def _pin(eng, bi):
                # Pin each engine's stream to emission order; prevents
                # scheduler priority inversions (engines execute in-order).
                if eng in _last:
                    _br.add_dep_helper(bi.ins, _last[eng].ins, sync=False,
                                       reason=f"{eng} emission order")
                _last[eng] = bi
                return bi

            def vop(bi):
                return _pin("v", bi)

            def aop(bi):
                return _pin("a", bi)

            def gop(bi):
                return _pin("g", bi)

            def pe(bi):
                return _pin("pe", bi)

            # ---- GEMM pieces per chunk ----
            gates = {}   # ci -> (XR, XZ, XH) sbuf tiles [128, tc*P] fp16

            def make_pieces(ci):
                """Returns (loads, eps_act, eps_dve): closures for chunk ci's
                x load and (gate,hb) matmul+epilogue groups split by the
                engine that runs the epilogue."""
                tcc = CSIZES[ci]
                cb = tcc * BC
                co = COFFS[ci] * BC
                # pool tags need constant shapes: allocate steady-size,
                # slice for the short head chunks
                XR = gpool.tile([128, TCO * P], fp16, tag="XR",
                                name=f"XR{ci}")[:, :tcc * P]
                XZ = gpool.tile([128, TCO * P], fp16, tag="XZ",
                                name=f"XZ{ci}")[:, :tcc * P]
                XH = gpool.tile([128, TCO * P], fp16, tag="XH",
                                name=f"XH{ci}")[:, :tcc * P]
                gates[ci] = (XR, XZ, XH)
                xk = [xkpool.tile([128, TCO * BC], fp16, tag=f"xk{k}",
                                  name=f"xk{k}_{ci}")[:, :cb]
                      for k in range(KT)]

                def load(k, xk=xk):
                    # 2D view: one contiguous run per partition
                    nc.sync.dma_start(
                        out=xk[k],
                        in_=xT_d.rearrange("(k p) n b -> k p (n b)", p=128)
                        [k, :, co:co + cb])
                loads = [lambda k=k: load(k) for k in range(KT)]

                def scale_bias(g):
                    scale = 0.5 if g == "h" else 1.0
                    if g == "r":
                        bias = -1.0 if fast else brv
                    elif g == "z":
                        bias = 0.0 if fast else bzv
                    else:
                        bias = 0.0
                    return scale, bias

                def mmpair(g, hb, dest, on_dve=False, tcc=tcc, cb=cb, xk=xk,
                           ci=ci):
                    # two hb-halves into one psum tile, ONE wide epilogue
                    ps = pspool.tile([128, 2 * TCO * BC], fp32, tag="mm",
                                     name=f"mm{ci}_{g}{hb}")
                    for half in range(2):
                        for k in range(KT):
                            pe(nc.tensor.matmul(
                                out=ps[:, half * cb:half * cb + cb],
                                lhsT=w_sb[(g, k)][:, (hb + half) * 128:
                                                  (hb + half + 1) * 128],
                                rhs=xk[k],
                                start=(k == 0), stop=(k == KT - 1)))
                    dst = dest.rearrange(
                        "p (t hb b) -> p hb t b", t=tcc, hb=HB)[:, hb:hb + 2]
                    ps4 = ps[:, :2 * cb].rearrange(
                        "p (i t b) -> p i t b", i=2, t=tcc)
                    scale, bias = scale_bias(g)
                    if g == "r" and not fast:
                        bias = brv[:, hb:hb + 1]  # not pair-safe in general
                    if on_dve:   # chunk-0 priming: keep ACT free for s_0
                        sc2 = bias if isinstance(bias, float) else 0.0
                        vop(nc.vector.tensor_scalar(
                            out=dst, in0=ps4, scalar1=scale, scalar2=sc2,
                            op0=OP.mult, op1=OP.add))
                    else:
                        aop(nc.scalar.activation(
                            out=dst, in_=ps4, func=AF.Identity,
                            bias=bias, scale=scale))

                def mmsingle(g, hb, dest, on_dve, tcc=tcc, cb=cb, xk=xk,
                             ci=ci):
                    ps = pspools.tile([128, TCO * BC], fp32, tag="mms",
                                      name=f"mms{ci}_{g}{hb}")
                    for k in range(KT):
                        pe(nc.tensor.matmul(
                            out=ps[:, :cb],
                            lhsT=w_sb[(g, k)][:, hb * 128:(hb + 1) * 128],
                            rhs=xk[k],
                            start=(k == 0), stop=(k == KT - 1)))
                    dst = dest.rearrange(
                        "p (t hb b) -> p t hb b", t=tcc, hb=HB)[:, :, hb, :]
                    ps3 = ps[:, :cb].rearrange("p (t b) -> p t b", t=tcc)
                    scale, bias = scale_bias(g)
                    if g == "r" and not fast:
                        bias = brv[:, hb:hb + 1]
                    elif g == "z" and not fast:
                        bias = bzv[:, hb:hb + 1]
                    if on_dve:
                        sc2 = bias if isinstance(bias, float) else 0.0
                        vop(nc.vector.tensor_scalar(
                            out=dst, in0=ps3, scalar1=scale, scalar2=sc2,
                            op0=OP.mult, op1=OP.add))
                    else:
                        aop(nc.scalar.activation(
                            out=dst, in_=ps3, func=AF.Identity,
                            bias=bias, scale=scale))

                eps_act = []
                eps_dve = []
                if tcc < TCO or not fast:
                    # head / general chunks: all singles; h-gate on DVE in
                    # the fast path to keep ACT's queue short at startup
                    for g, dest in (("r", XR), ("z", XZ), ("h", XH)):
                        for hb in range(HB):
                            dve = fast and g == "h"
                            (eps_dve if dve else eps_act).append(
                                lambda g=g, hb=hb, dest=dest, dve=dve:
                                mmsingle(g, hb, dest, dve))
                else:
                    # steady chunks: 12 pair-epilogues (ACT; chunk-0 priming
                    # overrides z/h onto DVE via on_dve)
                    for g, dest in (("r", XR), ("z", XZ), ("h", XH)):
                        for hb in range(0, HB, 2):
                            eps_act.append(
                                lambda g=g, hb=hb, dest=dest, on_dve=False:
                                mmpair(g, hb, dest, on_dve=on_dve))
                return loads, eps_act, eps_dve

            def emit_output(ci, ring, lo, hi):
                """xbar-transpose steps [lo,hi) of chunk ci's hm ring and
                DMA to ys."""
                nt = hi - lo
                stg = stpool.tile([128, nt * P], fp16, tag="stg",
                                  name=f"stg{ci}_{lo}")
                nc.sync.dma_start_transpose(
                    out=stg.rearrange("p (g m) -> p g m", m=128),
                    in_=ring[:, lo * P:hi * P].rearrange(
                        "p (g u) -> p g u", u=128))
                ot0 = COFFS[ci] - W + lo
                dst = ys_d[:, :, ot0:ot0 + nt, :, :].rearrange(
                    "l b t j c -> (l b) t j c")
                nc.sync.dma_start(
                    out=dst,
                    in_=stg.rearrange("p (t j c) -> p t j c", t=nt, j=JD))

            # ---- emit: prime chunk 0 (x first, then weights, then r-gate
            # epilogues on ACT and z/h on DVE so nothing queues ahead of the
            # first sigmoids) ----
            pend_act = []
            pend_dve = []
            loads0, eps_act0, eps_dve0 = make_pieces(0)
            for p_ in loads0:
                p_()
            # weights on the scalar-engine HWDGE queue: they issue and
            # transfer concurrently with the chunk-0/1 x loads on the sync
            # queue instead of serializing behind them (r-gate first)
            for wt, wd in w_dma:
                nc.scalar.dma_start(
                    out=wt.rearrange("p (k h) -> p k h", k=KT),
                    in_=wd.rearrange("(k p) h -> p k h", p=128))
            if not fast:
                nc.sync.dma_start(out=brv, in_=brv_d[:, :])
            if fast:
                NP = HB // 2          # pair-groups per gate (4)
                for p_ in eps_act0[:NP]:   # r-pairs -> ACT (feed s_0)
                    p_()
                # z-pairs and half the h-pairs -> DVE; the other h-pairs run
                # on ACT in parallel so the h-gate epilogue tail (the head's
                # binding path after chunk-0's GEMM) halves
                for j, p_ in enumerate(eps_act0[NP:]):
                    if j >= 2 * NP - 2:
                        p_()
                    else:
                        p_(on_dve=True)
            else:
                for p_ in eps_act0:
                    p_()
            for p_ in eps_dve0:
                p_()

            hh, hm = hh0, hm0

            def s_tile(tag, i):
                return spool.tile([128, P], fp16, tag=tag, name=f"{tag}_{i}")

            def gate_col(gt, t):
                return gt[:, t * P:(t + 1) * P]

            # a_0 / c_0
            XR, XZ, XH = gates[0]
            a_t = s_tile("a", 0)
            c_t = s_tile("c", 0)
            if fast:
                vop(nc.vector.tensor_tensor(a_t, gate_col(XR, 0), hh, OP.add))
                vop(nc.vector.tensor_tensor(c_t, gate_col(XZ, 0), hm, OP.add))
            else:
                t1 = s_tile("t1", 0)
                vop(nc.vector.tensor_tensor(t1, mrt, hm, OP.mult))
                vop(nc.vector.tensor_tensor(a_t, t1, gate_col(XR, 0), OP.add))
                t2 = s_tile("t2", 0)
                vop(nc.vector.tensor_tensor(t2, mzt, hm, OP.mult))
                vop(nc.vector.tensor_tensor(c_t, t2, gate_col(XZ, 0), OP.add))

            for ci in range(NCH):
                tcc = CSIZES[ci]
                XR, XZ, XH = gates[ci]
                if ci + 1 < NCH:
                    loads, eps_a, eps_d = make_pieces(ci + 1)
                    for p_ in loads:
                        p_()
                    pend_act.extend(eps_a)
                    pend_dve.extend(eps_d)
                nsteps_left = tcc
                ring = (rpool.tile([128, TCO * P], fp16, tag="ring",
                                   name=f"ring{ci}") if ci >= OC0 else None)
                for t in range(tcc):
                    i = COFFS[ci] + t
                    last = (i == N - 1)
                    if fast and tcc == TCO and nsteps_left > 1:
                        # alternating 1/2 keeps ACT's epilogue backlog (and
                        # so the sigmoid queue delay) minimal vs ceil-spread
                        na = min(len(pend_act), 1 if t % 2 == 0 else 2)
                    else:
                        na = -(-len(pend_act) // max(nsteps_left, 1))
                    nd = (-(-len(pend_dve) // nsteps_left)
                          if len(pend_dve) >= nsteps_left else 0)
                    nsteps_left -= 1
                    # chain front: s, sh, e3, q  (a_t from previous tail)
                    s_ = s_tile("s", i)
                    aop(nc.scalar.activation(s_, a_t, AF.Sigmoid, scale=2.0))
                    sh = s_tile("sh", i)
                    vop(nc.vector.tensor_tensor(sh, hm, s_, OP.mult))
                    e3 = s_tile("e3", i)
                    vop(nc.vector.tensor_tensor(e3, sh, gate_col(XH, t),
                                                OP.add))
                    z_ = s_tile("z", i)
                    aop(nc.scalar.activation(z_, c_t, AF.Sigmoid))
                    q_ = s_tile("q", i)
                    aop(nc.scalar.activation(q_, e3, AF.Sigmoid, scale=4.0))
                    # epilogue groups, in ACT's post-q window
                    for _ in range(na):
                        if pend_act:
                            pend_act.pop(0)()
                    for _ in range(nd):
                        if pend_dve:
                            pend_dve.pop(0)()
                    # off-chain tail
                    U2 = s_tile("U2", i)
                    vop(nc.vector.tensor_scalar(
                        out=U2, in0=z_, scalar1=-2.0, scalar2=2.0,
                        op0=OP.mult, op1=OP.add))
                    hz1 = s_tile("hz1", i)
                    vop(nc.vector.tensor_tensor(hz1, hh, z_, OP.mult))
                    v_ = s_tile("vv", i)
                    vop(nc.vector.tensor_tensor(v_, q_, U2, OP.mult))
                    hh_n = s_tile("hh", i)
                    vop(nc.vector.tensor_tensor(hh_n, v_, hz1, OP.add))
                    # chain-critical a' right after hh'
                    if not last and fast:
                        a_t = s_tile("a", i + 1)
                        XRn = gates[ci + 1][0] if t == tcc - 1 else XR
                        vop(nc.vector.tensor_tensor(
                            a_t, hh_n, gate_col(XRn, (t + 1) % tcc
                                                if t == tcc - 1 else t + 1),
                            OP.add))
                    hm_n = (ring[:, t * P:(t + 1) * P] if ring is not None
                            else s_tile("hm", i))
                    vop(nc.vector.tensor_scalar(
                        out=hm_n, in0=hh_n, scalar1=-1.0, scalar2=None,
                        op0=OP.add))
                    if not last:
                        c_t = s_tile("c", i + 1)
                        tn = 0 if t == tcc - 1 else t + 1
                        if fast:
                            XZn = gates[ci + 1][1] if t == tcc - 1 else XZ
                            vop(nc.vector.tensor_tensor(
                                c_t, gate_col(XZn, tn), hm_n, OP.add))
                        else:
                            a_t = s_tile("a", i + 1)
                            XRn = gates[ci + 1][0] if t == tcc - 1 else XR
                            XZn = gates[ci + 1][1] if t == tcc - 1 else XZ
                            t1 = s_tile("t1", i + 1)
                            vop(nc.vector.tensor_tensor(t1, mrt, hm_n,
                                                        OP.mult))
                            vop(nc.vector.tensor_tensor(
                                a_t, t1, gate_col(XRn, tn), OP.add))
                            t2 = s_tile("t2", i + 1)
                            gop(nc.gpsimd.tensor_tensor(t2, mzt, hm_n,
                                                        OP.mult))
                            gop(nc.gpsimd.tensor_tensor(
                                c_t, t2, gate_col(XZn, tn), OP.add))
                    hh = hh_n
                    hm = hm_n
                    # split the last chunk's output to shorten the tail
                    if (ring is not None and ci == NCH - 1
                            and t == tcc // 2 - 1):
                        emit_output(ci, ring, 0, tcc // 2)
                if ring is not None:
                    if ci == NCH - 1:
                        emit_output(ci, ring, tcc // 2, tcc)
                    else:
                        emit_output(ci, ring, 0, tcc)

    _legalize_sync_waits(nc)
    return nc


def _get_nc(fast: bool):
    if fast not in _cache:
        _cache[fast] = _build(fast)
    return _cache[fast]


LAST_RESULT = None


def kernel(**inputs):
    global LAST_RESULT
    from concourse.bass_utils import run_bass_kernel_spmd

    x = np.asarray(inputs["x"], dtype=np.float32)
    kz = np.asarray(inputs["kz"], dtype=np.float32)
    kr = np.asarray(inputs["kr"], dtype=np.float32)
    kh = np.asarray(inputs["kh"], dtype=np.float32)
    mz = np.asarray(inputs["mz"], dtype=np.float32)
    mr = np.asarray(inputs["mr"], dtype=np.float32)
    br = np.asarray(inputs["br"], dtype=np.float32)
    bz = np.asarray(inputs["bz"], dtype=np.float32)
    assert x.shape == (B, T, D) and kz.shape == (D, H)

    fast = bool(np.all(mz == 1.0) and np.all(mr == 1.0)
                and np.all(br == 0.0) and np.all(bz == 0.0)
                and not int(os.environ.get("FORCE_SLOW", "0")))
    nc = _get_nc(fast)

    def pvec(v):  # [H] -> [128, HB] with [h_a, h_b]
        return np.ascontiguousarray(v.reshape(HB, 128).T)

    def ptile(v):  # [H] -> [128, (hb, b)] fp16, replicated over b
        t = v.reshape(HB, 128).T
        return np.ascontiguousarray(
            np.repeat(t[:, :, None], BC, axis=2).reshape(128, P)
        ).astype(np.float16)

    base = {
        "kz": np.ascontiguousarray(kz).astype(np.float16),
        "kr": np.ascontiguousarray(kr).astype(np.float16),
        "kh": np.ascontiguousarray(kh).astype(np.float16),
        "brv": pvec((br - mr) if fast else br).astype(np.float32),
    }
    if not fast:
        base["bzv"] = pvec(bz).astype(np.float32)
        base["mrt"] = ptile(mr)
        base["mzt"] = ptile(mz)

    x16 = x.astype(np.float16)
    in_maps = []
    for i in range(NCORES):
        i_t, i_b = i // SB, i % SB
        t0 = i_t * SEG
        bs = slice(i_b * BC, (i_b + 1) * BC)
        xc = np.zeros((BC, N, D), np.float16)
        src = x16[bs, max(0, t0 - W):t0 + SEG]
        xc[:, N - src.shape[1]:, :] = src
        xTc = np.ascontiguousarray(xc.transpose(2, 1, 0))
        in_maps.append(dict(base, xT=xTc))

    trace = bool(int(os.environ.get("KERNEL_TRACE", "0")))
    res = run_bass_kernel_spmd(nc, in_maps, list(range(NCORES)), trace=trace)
    LAST_RESULT = res
    ys = np.empty((B, T, H), np.float32)
    for i in range(NCORES):
        i_t, i_b = i // SB, i % SB
        yc = res.results[i]["ys"].astype(np.float32)  # [l, b, t, j, c]
        ys[i_b * BC:(i_b + 1) * BC, i_t * SEG:(i_t + 1) * SEG, :] = (
            yc.transpose(1, 2, 3, 0, 4).reshape(BC, SEG, H))
    return ys
